# revision 56
# baseline (speedup 1.0000x reference)
"""Trainium2 Bass kernel for the DGL ChildSum-TreeLSTM problem.

Strategy (per spec sharding hint): 32 independent trees -> 4 trees per
NeuronCore, weights replicated, logits gathered on host.

Per-core computation is restructured around a host-computed "slot layout":
every level's nodes are permuted so that the children of each 128-parent
tile sit in a contiguous, chunk-aligned window of the next level's node
order.  The irregular-fan-in segment_sum then becomes a handful of
128x128 one-hot selector matmuls accumulating in PSUM, and all LSTM state
stays resident in SBUF in fp16 (no DRAM traffic for states at all).

Key optimizations (in order of impact):
 - **Leaf folding**: a leaf's (h, c) is a pure function of its token, so
   the whole leaf level (half of all nodes) is precomputed on the host
   into a [VOCAB, 512] fp16 table [h | c~] and row-gathered on device --
   no leaf-level compute at all.
 - The x @ W_iou^T product for non-leaf nodes is folded into the
   embedding table on the host (emb_iou, fp16, pre-scaled by SCALE), so
   their only data-dependent DMA is a row gather of iou preactivations.
 - The u-slice of W/U/b is pre-scaled by 2 on the host and c~ = c/2 is
   tracked on device, which turns tanh(u) into an affine image of
   sigmoid(2u): ONE sigmoid per node tile covers i, o and u, and
   tanh(c) = tanh(2*c~) is an activation-scale.  c~ stays exact through
   the linear child-sum recursion.
 - U_f runs as an fp8e4m3 DoubleRow matmul (256-contraction in one
   instruction at 0.5 cycles/row); weights carry x SU and the
   transposed-h operand x SH, undone by the f-sigmoid's 1/SCALE.  The
   f-gate path tolerates fp8 (~5e-4 rel err); U_iou and the x path do
   NOT (tested ~3e-2) and stay fp16.
 - Engine assignment follows per-PHASE occupancy: during the leaf /
   level-10 startup Pool is busy with SWDGE descriptor generation, so
   leaf fc-muls and fp8 conversions run on the otherwise-idle DVE;
   for levels 8-9 DVE is the bottleneck, so fc/h-muls go to GPSIMD.
 - tanh(c) is batched over quads of node tiles; selector matrices are
   fetched in 12-window chunks, small-level selectors and emb_iou rows
   in one hoisted prefetch emitted mid-pipeline (level 9) to keep the
   startup DMA window clear for the leaf-table gathers.
"""

import numpy as np
import ml_dtypes

import concourse.bacc as bacc
import concourse.bass as bass  # noqa: F401
import concourse.mybir as mybir
import concourse.tile as tile
from concourse.bass_utils import run_bass_kernel_spmd
from concourse.masks import make_identity

# ---- static problem structure (from the reference nn.Module) ----
B = 32
DEPTH = 12
LEVELS = [2 ** d for d in range(DEPTH)]
_STARTS = [0]
for _l in LEVELS[:-1]:
    _STARTS.append(_STARTS[-1] + _l)
NPG = sum(LEVELS)              # 4095 nodes per tree
N = B * NPG
VOCAB = 20000
PAD = VOCAB - 1
E = 256
H = 256
NCLS = 104
NCORES = 8
TPC = B // NCORES              # trees per core
F8 = mybir.dt.float8e4
F16 = mybir.dt.float16
F32 = mybir.dt.float32
I16 = mybir.dt.int16
GATHER_GROUP = 6               # node tiles per xsm prefetch gather
XGG = 6                        # node tiles per emb_iou gather group
USE_HT_GATHER = False           # fetch leaf h pre-transposed via dma_gather
SU = 16.0                      # fp8 U_iou / U_f weight scale
SH = 8.0                       # fp8 transposed-h operand scale
SCALE = SU * SH                # combined PSUM preact scale
SIG = mybir.ActivationFunctionType.Sigmoid
TANH = mybir.ActivationFunctionType.Tanh
COPYF = mybir.ActivationFunctionType.Copy
DR = mybir.MatmulPerfMode.DoubleRow
ADD = mybir.AluOpType.add
MULT = mybir.AluOpType.mult


# --------------------------------------------------------------------------
# host-side slot layout
# --------------------------------------------------------------------------

def _core_layout(x, par, core):
    """Per-core slot assignment. Returns per level: node-id per slot and the
    window size m (see module docstring)."""
    trees = range(core * TPC, (core + 1) * TPC)
    slot_nodes = [np.array([g * NPG for g in trees], dtype=np.int64)]
    levels = []
    for d in range(DEPTH):
        nodes = slot_nodes[d]
        Lpad = ((len(nodes) + 127) // 128) * 128
        if Lpad > len(nodes):
            nodes = np.concatenate([nodes, np.full(Lpad - len(nodes), -1,
                                                   np.int64)])
        slot_nodes[d] = nodes
        T = Lpad // 128
        lv = {"nodes": nodes, "T": T, "L": Lpad, "m": 0}
        levels.append(lv)
        if d == DEPTH - 1:
            continue
        # children of every real node in this level, grouped by parent
        ch_by_parent = {}
        for g in trees:
            lo = g * NPG + _STARTS[d + 1]
            hi = lo + LEVELS[d + 1]
            p = np.asarray(par[lo:hi])
            order = np.argsort(p, kind="stable")
            ids = np.arange(lo, hi, dtype=np.int64)[order]
            ps = p[order]
            uniq, start_idx = np.unique(ps, return_index=True)
            bounds = list(start_idx) + [len(ps)]
            for i, u in enumerate(uniq):
                ch_by_parent[int(u)] = ids[bounds[i]:bounds[i + 1]]
        cursor = 0
        child_slots = []
        m = 2
        for t in range(T):
            start = max(256 * t, cursor)
            child_slots.extend([-1] * (start - cursor))
            kids = []
            for s in range(128):
                node = nodes[t * 128 + s]
                if node >= 0:
                    kids.extend(ch_by_parent.get(int(node), ()))
            child_slots.extend(kids)
            cursor = start + len(kids)
            if cursor > 256 * t:
                m = max(m, -(-(cursor - 256 * t) // 128))
        lv["m"] = m
        slot_nodes.append(np.array(child_slots, dtype=np.int64))
    return levels


def build_layouts(x, par):
    x = np.asarray(x)
    par = np.asarray(par)
    cores = [_core_layout(x, par, c) for c in range(NCORES)]
    Ls = [max(cores[c][d]["L"] for c in range(NCORES)) for d in range(DEPTH)]
    Ts = [L // 128 for L in Ls]
    ms = [max(cores[c][d]["m"] for c in range(NCORES)) for d in range(DEPTH)]
    out = {"L": Ls, "T": Ts, "m": ms, "cores": []}
    for c in range(NCORES):
        lvs = cores[c]
        node_slots = []
        xtok = []
        for d in range(DEPTH):
            nodes = lvs[d]["nodes"]
            if len(nodes) < Ls[d]:
                nodes = np.concatenate(
                    [nodes, np.full(Ls[d] - len(nodes), -1, np.int64)])
            node_slots.append(nodes)
            tok = np.where(nodes >= 0, np.asarray(x)[np.maximum(nodes, 0)],
                           PAD)
            xtok.append(tok.astype(np.int16))
        relslot = []
        for d in range(DEPTH - 1):
            T, m = Ts[d], ms[d]
            child_nodes = node_slots[d + 1]
            rel = np.full((T, m, 128), -1.0, np.float16)
            pslot_of = {int(n): i for i, n in enumerate(node_slots[d])
                        if n >= 0}
            nch = Ls[d + 1] // 128
            for j in range(nch):
                for p in range(128):
                    node = child_nodes[j * 128 + p]
                    if node < 0:
                        continue
                    ps = pslot_of[int(par[node])]
                    t, sl = ps // 128, ps % 128
                    w = j - 2 * t
                    assert 0 <= w < m
                    rel[t, w, p] = sl
            relslot.append(rel)
        out["cores"].append({"xtok": xtok, "relslot": relslot,
                             "nodes": node_slots})
    return out


# --------------------------------------------------------------------------
# device program
# --------------------------------------------------------------------------

def _wrap16(tok):
    """int16 [L] -> the dma_gather 16-partition wrapped layout [16, L//16]."""
    return tok.reshape(-1, 16).T.copy()


def _ilhs(gt_ap, ni, off):
    """DoubleRow lhsT view [128, 2, 128] into a [128, 2, ni] fp8
    transpose-gather tile.  The gather writes, per partition p, an
    idx-major byte stream: byte q holds dim (2p + q%2) of idx q//2, so
    nodes off..off+127 occupy the 256 contiguous bytes at 2*off; the DR
    k-tile axis is the byte parity."""
    jj, oo = divmod(2 * off, ni)
    return gt_ap[:, jj, oo:oo + 256].rearrange("p (m b) -> p b m", b=2)


def build_program(Ls, Ts, ms, with_biou, with_ufb, with_linb):
    nch_of = [Ls[d] // 128 for d in range(DEPTH)]
    idx_cols = [Ls[d] // 16 for d in range(DEPTH)]
    idx_off = np.concatenate([[0], np.cumsum(idx_cols)]).astype(int)
    sel_cnt = [Ts[d] * ms[d] for d in range(DEPTH - 1)]
    sel_off = np.concatenate([[0], np.cumsum(sel_cnt)]).astype(int)
    nsel = int(sel_off[-1])

    nc = bacc.Bacc("TRN2", debug=False, num_devices=NCORES)

    emb8 = nc.dram_tensor("emb8", [VOCAB, 3 * H], F16,
                          kind="ExternalInput").ap()
    hc8 = nc.dram_tensor("hc8", [VOCAB, 2 * H], F16,
                         kind="ExternalInput").ap()
    ht8 = nc.dram_tensor("ht8", [VOCAB, H], F8,
                         kind="ExternalInput").ap()
    ufT_il = nc.dram_tensor("ufT_il", [128, 2, H], F8,
                            kind="ExternalInput").ap()
    uiou8T = nc.dram_tensor("uiou8T", [2, 128, 3 * H], F16,
                            kind="ExternalInput").ap()
    ufT = nc.dram_tensor("ufT", [2, 128, H], F8,
                         kind="ExternalInput").ap()
    linT = nc.dram_tensor("linT", [2, 128, NCLS], F16,
                          kind="ExternalInput").ap()
    biases = nc.dram_tensor("biases", [1, 3 * H + H + NCLS], F16,
                            kind="ExternalInput").ap()
    xtok = nc.dram_tensor("xtok", [128, int(idx_off[-1])], I16,
                          kind="ExternalInput").ap()
    selmat = nc.dram_tensor("selmat", [128, max(nsel, 1), 128], F16,
                            kind="ExternalInput").ap()
    logits_t = nc.dram_tensor("logits_t", [NCLS, 128], F32,
                              kind="ExternalOutput").ap()

    with tile.TileContext(nc) as tc:
        with (
            tc.tile_pool(name="const", bufs=1) as cpool,
            tc.tile_pool(name="state", bufs=1) as spool,
            tc.tile_pool(name="hcg", bufs=7) as hcpool,
            tc.tile_pool(name="hT", bufs=3) as htpool,
            tc.tile_pool(name="xg", bufs=2) as xpool,
            tc.tile_pool(name="sio", bufs=7) as siopool,
            tc.tile_pool(name="sel", bufs=4) as selpool,
            tc.tile_pool(name="work", bufs=6) as wpool,
            tc.tile_pool(name="chunk", bufs=4) as chpool,
            tc.tile_pool(name="psel", bufs=2, space="PSUM") as psel_pool,
            tc.tile_pool(name="p256", bufs=2, space="PSUM") as p256_pool,
            tc.tile_pool(name="piou", bufs=2, space="PSUM") as piou_pool,
        ):
            # ---- resident constants ----
            uiou_sb = cpool.tile([128, 2, 3 * H], F16)
            uf_sb = cpool.tile([128, 2, H], F8)
            uf_il_sb = cpool.tile([128, 2, H], F8)
            nc.sync.dma_start(uf_il_sb[:], ufT_il[:])
            lin_sb = cpool.tile([128, 2, NCLS], F16)
            bias_sb = cpool.tile([1, 3 * H + H + NCLS], F16)
            ones_sb = cpool.tile([1, 128], F16)
            idx_sb = cpool.tile([128, int(idx_off[-1])], I16)
            nc.sync.dma_start(idx_sb[:], xtok[:])
            for k in range(2):
                nc.sync.dma_start(uiou_sb[:, k, :], uiou8T[k])
                nc.sync.dma_start(uf_sb[:, k, :], ufT[k])
                nc.sync.dma_start(lin_sb[:, k, :], linT[k])
            nc.sync.dma_start(bias_sb[:], biases[:])
            nc.vector.memset(ones_sb[:], 1.0)
            ident = cpool.tile([128, 128], F16)
            make_identity(nc, ident[:])

            # ---- hoisted prefetch for the small top levels ----
            SMALL_IDX = 1536   # first 1536 slots = levels 0..7 here
            small_set = set(d for d in range(DEPTH)
                            if int(idx_off[d + 1]) * 16 <= SMALL_IDX)
            n_sm_g = (min(SMALL_IDX, int(idx_off[-1]) * 16)
                      + 767) // 768
            xsm = []
            nsel_sm = int(sel_off[max(small_set) + 1]) if small_set else 0
            s3sm = (cpool.tile([128, nsel_sm, 128], F16, name="s3sm")
                    if nsel_sm else None)

            def small_prefetch():
                """Emitted after the leaf level so these bulk transfers do
                not contend with the first leaf gathers on the DMA engines."""
                for g in range(n_sm_g):
                    xs = cpool.tile([128, GATHER_GROUP, 3 * H], F16,
                                    name=f"xsm{g}")
                    nc.gpsimd.dma_gather(
                        xs[:, :, :], emb8[:],
                        idx_sb[:, g * 48:(g + 1) * 48],
                        768, 768, 3 * H, transpose=False)
                    xsm.append(xs)
                if nsel_sm:
                    nc.sync.dma_start(s3sm[:], selmat[:, 0:nsel_sm, :])

            state = {}   # level -> (h quads, c quads), each [128, 4, H]
            SELCH = 12   # selectors per fetch chunk

            def transpose256(src_ap, tag):
                """[128,256] fp16 -> [128,256] fp16 holding the two
                transposed 128x128 halves side by side."""
                pt = p256_pool.tile([128, 256], F16, tag="p256", name="pt")
                nc.tensor.transpose(pt[:, 0:128], src_ap[:, 0:128], ident[:])
                nc.tensor.transpose(pt[:, 128:256], src_ap[:, 128:256],
                                    ident[:])
                dst = wpool.tile([128, 256], F16, tag=tag, bufs=1,
                                 name=tag)
                nc.vector.tensor_copy(dst[:], pt[:])
                return dst

            def matmul_group(out_ap, pairs):
                """Emit an accumulation group into one psum zero-region."""
                for i, (lhsT, rhs) in enumerate(pairs):
                    nc.tensor.matmul(out_ap, lhsT, rhs, start=(i == 0),
                                     stop=(i == len(pairs) - 1))

            # state slots per parity class: enough quad-tiles for the
            # largest level of that parity (the leaf level has no state
            # tiles: its (h, c~) arrives pre-computed via dma_gather)
            state_bufs = [0, 0]
            for d in range(DEPTH - 1):
                state_bufs[d % 2] = max(state_bufs[d % 2],
                                        (Ts[d] + 3) // 4)

            HCG = 6              # leaf hc row-gather group (tiles)
            HTCH = 2048          # idx per transposed-h gather chunk

            for d in range(DEPTH - 1, -1, -1):
                L, T, m = Ls[d], Ts[d], ms[d]
                if d == DEPTH - 1:
                    # ---- leaf level: (h, c~) gathered from the host-folded
                    # per-token table; h additionally gathered transposed
                    # (feature-major) for the U_f lhsT of the level above.
                    lbase = int(idx_off[d])
                    hcg, hTc = {}, {}

                    def issue_hcg(g, hcg=hcg, lbase=lbase, T=T):
                        if g < 0 or g >= -(-T // HCG) or g in hcg:
                            return
                        gt = min(HCG, T - g * HCG)
                        ht_ = hcpool.tile([128, gt, 2 * H], F16,
                                          tag="hcg", name="hcg")
                        nc.gpsimd.dma_gather(
                            ht_[:, :, :], hc8[:],
                            idx_sb[:, lbase + g * HCG * 8:
                                   lbase + g * HCG * 8 + gt * 8],
                            gt * 128, gt * 128, 2 * H, transpose=False)
                        hcg[g] = ht_

                    def issue_hT(k, hTc=hTc, lbase=lbase, T=T):
                        if k < 0 or k >= -(-(T * 128) // HTCH) or k in hTc:
                            return
                        ni = min(HTCH, T * 128 - k * HTCH)
                        tt_ = htpool.tile([128, 2, ni], F8, tag="hT",
                                          name="hT")
                        nc.gpsimd.dma_gather(
                            tt_[:, :, :], ht8[:],
                            idx_sb[:, lbase + k * (HTCH // 16):
                                   lbase + (k * HTCH + ni) // 16],
                            ni, ni, H, transpose=True)
                        hTc[k] = tt_

                    issue_hcg(0)
                    if USE_HT_GATHER:
                        issue_hT(0)
                    issue_hcg(1)
                    if not USE_HT_GATHER:
                        def issue_hT(k):  # noqa: F811
                            pass
                    state[d] = ("leaf", hcg, hTc, issue_hcg, issue_hT)
                    continue
                if d == 9:
                    small_prefetch()
                nquad = (T + 3) // 4
                h_lv = [spool.tile([128, 4, H], F16, tag=f"h{d % 2}",
                                   bufs=state_bufs[d % 2],
                                   name=f"h{d}_{q}") for q in range(nquad)]
                c_lv = [spool.tile([128, 4, H], F16, tag=f"c{d % 2}",
                                   bufs=state_bufs[d % 2],
                                   name=f"c{d}_{q}") for q in range(nquad)]
                sel_ch = {}
                nselch = (-(-(T * m) // SELCH)
                          if (d < DEPTH - 1 and d not in small_set) else 0)

                def issue_selch(ci, d=d, sel_ch=sel_ch, nselch=nselch):
                    if ci >= nselch or ci in sel_ch:
                        return
                    cnt = min(SELCH, T * m - ci * SELCH)
                    st = selpool.tile([128, cnt, 128], F16, tag="sel",
                                      name="selch")
                    base = int(sel_off[d]) + ci * SELCH
                    nc.sync.dma_start(st[:], selmat[:, base:base + cnt, :])
                    sel_ch[ci] = st
                if d < DEPTH - 1:
                    child = state[d + 1]
                    leafch = isinstance(child, tuple) and child[0] == "leaf"
                    nch = nch_of[d + 1]
                    if leafch:
                        _, hcg, hTc, issue_hcg, issue_hT = child

                        def h_half(j, k, hcg=hcg):
                            return hcg[j // HCG][:, j % HCG,
                                                 k * 128:(k + 1) * 128]

                        def h_full(j, hcg=hcg):
                            return hcg[j // HCG][:, j % HCG, 0:H]

                        def c_rows(p, n, hcg=hcg):
                            g, r = (2 * p) // HCG, (2 * p) % HCG
                            return hcg[g][:, r:r + n, H:2 * H]

                        def hT_lhs(j, hTc=hTc):
                            ck, off = divmod(j * 128, HTCH)
                            ni = min(HTCH, Ls[DEPTH - 1] - ck * HTCH)
                            return _ilhs(hTc[ck][:, :, :], ni, off)
                    else:
                        h_ch, c_ch = child

                        def h_half(j, k, h_ch=h_ch):
                            return h_ch[j // 4][:, j % 4,
                                               k * 128:(k + 1) * 128]

                        def h_full(j, h_ch=h_ch):
                            return h_ch[j // 4][:, j % 4, :]

                        def c_rows(p, n, c_ch=c_ch):
                            q, r = (2 * p) // 4, (2 * p) % 4
                            return c_ch[q][:, r:r + n, :]

                        hT_lhs = None
                    chunk_pt = {}   # pair -> (hjT2, js_p) after stage A
                    chunk_fc = {}   # chunk -> fc AP after stage B

                    def stageA(p):
                        """PE transposes of the pair's h + psum->sbuf copy.
                        For leaf children h^T arrives via the transposed
                        gather, so the stage is dependency-tracking only."""
                        js_p = [j for j in (2 * p, 2 * p + 1) if j < nch]
                        if not js_p:
                            return
                        if leafch and USE_HT_GATHER:
                            chunk_pt[p] = (None, js_p)
                            return
                        w = len(js_p) * H
                        pt2 = p256_pool.tile([128, w], F16, tag="p256",
                                             name="pt2")
                        for ji, j in enumerate(js_p):
                            for k in range(2):
                                nc.tensor.transpose(
                                    pt2[:, ji * H + k * 128:
                                        ji * H + (k + 1) * 128],
                                    h_half(j, k), ident[:])
                        hjT2 = chpool.tile([128, 2 * len(js_p), 128], F8,
                                           tag="hjT", bufs=3, name="hjT2")
                        nc.vector.tensor_scalar_mul(hjT2[:], pt2[:], SH)
                        chunk_pt[p] = (hjT2, js_p)

                    def stageB(p):
                        """U_f fp8 DoubleRow matmul + sigmoid + f*c."""
                        if p not in chunk_pt:
                            return
                        hjT2, js_p = chunk_pt.pop(p)
                        w = len(js_p) * H
                        pf2 = p256_pool.tile([128, w], F32, tag="p256",
                                             name="pf2")
                        for ji, j in enumerate(js_p):
                            if hjT2 is None:
                                lhsT, rhs = hT_lhs(j), uf_il_sb[:, :, :]
                            else:
                                lhsT = hjT2[:, 2 * ji:2 * ji + 2, :]
                                rhs = uf_sb[:, :, :]
                            nc.tensor.matmul(
                                pf2[:, ji * H:(ji + 1) * H], lhsT, rhs,
                                perf_mode=DR, start=True,
                                stop=not with_ufb)
                            if with_ufb:
                                nc.tensor.matmul(
                                    pf2[:, ji * H:(ji + 1) * H], ones_sb[:],
                                    bias_sb[:, 3 * H:4 * H],
                                    start=False, stop=True)
                        fj2 = chpool.tile([128, w], F16, tag="fj",
                                          bufs=2, name="fj2")
                        nc.scalar.activation(fj2[:], pf2[:], SIG,
                                             scale=1.0 / SCALE)
                        fc2 = chpool.tile([128, w], F16, tag="fcj",
                                          bufs=6, name="fc2")
                        fc_eng = (nc.gpsimd if d in (8, 9)
                                  else nc.vector)
                        fc_eng.tensor_mul(fc2[:], fj2[:],
                                          c_rows(p, len(js_p)))
                        for ji, j in enumerate(js_p):
                            chunk_fc[j] = fc2[:, ji * H:(ji + 1) * H]

                # xio for this level: hoisted prefetch for small levels,
                # grouped dma_gathers (issued inside the t-loop) otherwise
                xg = {}

                def issue_gather(g, d=d, xg=xg):
                    if d in small_set or g * XGG >= T:
                        return
                    gt = min(XGG, T - g * XGG)
                    gi = gt * 128
                    xt = xpool.tile([128, gt, 3 * H], F16, tag="xt",
                                    name="xt")
                    nc.gpsimd.dma_gather(
                        xt[:, :, :], emb8[:],
                        idx_sb[:, int(idx_off[d]) + g * XGG * 8:
                               int(idx_off[d]) + g * XGG * 8
                               + gt * 8],
                        gi, gi, 3 * H, transpose=False)
                    xg[g] = xt

                if d in small_set:
                    eb = int(idx_off[d]) * 16

                    def xslice(t, eb=eb):
                        g, off = divmod(eb + t * 128, 768)
                        return xsm[g][:, off // 128, :]
                else:
                    def xslice(t, xg=xg):
                        return xg[t // XGG][:, t % XGG, :]

                def js_of(t):
                    return [2 * t + w for w in range(m) if 2 * t + w < nch]

                def pairs_of(t):
                    return sorted({j // 2 for j in js_of(t)})

                def Sw_of(t, wi):
                    if d in small_set:
                        return s3sm[:, int(sel_off[d]) + t * m + wi, :]
                    si = t * m + wi
                    return sel_ch[si // SELCH][:, si % SELCH, :]

                selsb_q = {}
                doneA, doneB = set(), set()

                def stagesAB(tA, tB):
                    if 0 <= tA < T:
                        for p in pairs_of(tA):
                            if p not in doneA:
                                stageA(p)
                                doneA.add(p)
                    if 0 <= tB < T:
                        for p in pairs_of(tB):
                            if p not in doneB:
                                stageB(p)
                                doneB.add(p)

                def stageC(t):
                    psel = psel_pool.tile([128, 2 * H], F32, tag="psel",
                                          name="psel")
                    js = js_of(t)
                    if js:
                        nmm = 3 * len(js)
                        k = 0
                        for wi, j in enumerate(js):
                            Sw = Sw_of(t, wi)
                            nc.tensor.matmul(
                                psel[:, 0:128], h_half(j, 0), Sw,
                                start=(k == 0), stop=(k == nmm - 1))
                            k += 1
                            nc.tensor.matmul(
                                psel[:, 128:256], h_half(j, 1),
                                Sw, start=(k == 0), stop=(k == nmm - 1))
                            k += 1
                            nc.tensor.matmul(
                                psel[:, 2 * 128:2 * 128 + H], Sw,
                                chunk_fc[j][:],
                                start=(k == 0), stop=(k == nmm - 1))
                            k += 1
                    else:
                        raise AssertionError("empty selector window")
                    # h~^T halves + c_agg -> fp16 SBUF (psel retires fast;
                    # the later c~ add then runs in the DVE 2x mode)
                    selsb8 = wpool.tile([128, 2, 128], F16, tag="s8",
                                        bufs=3, name="s8")
                    caggsb = wpool.tile([128, 256], F16, tag="cagg",
                                        bufs=3, name="cagg")
                    nc.vector.tensor_copy(selsb8[:], psel[:, 0:256])
                    nc.vector.tensor_copy(caggsb[:], psel[:, 256:512])
                    selsb_q[t] = (selsb8, caggsb[:])

                pend = []   # (t, sio_u) awaiting the quad tanh(c) + h mul

                def flush_pend(k):
                    grp = pend[:k]
                    del pend[:k]
                    t0 = grp[0][0]
                    npr = len(grp)
                    tcn = wpool.tile([128, npr, H], F16, tag="tcn",
                                     bufs=2, name="tcn")
                    nc.scalar.activation(tcn[:],
                                         c_lv[t0 // 4][:, 0:npr, :],
                                         TANH, scale=2.0)
                    mul_eng = nc.gpsimd if d in (8, 9) else nc.vector
                    for i, (tt, sio_t) in enumerate(grp):
                        mul_eng.tensor_mul(h_lv[tt // 4][:, tt % 4, :],
                                           sio_t[:, H:2 * H],
                                           tcn[:, i, :])

                def stageDE(t):
                    sio_u = siopool.tile([128, 3 * H], F16, tag="sio",
                                         name="sio")
                    selsb8, caggs = selsb_q.pop(t)
                    pio = piou_pool.tile([128, 768], F32, tag="pio",
                                         name="pio")
                    for r0, r1 in ((0, 512), (512, 768)):
                        nc.tensor.matmul(pio[:, r0:r1], ident[:],
                                         xslice(t)[:, r0:r1],
                                         start=True, stop=False)
                        nc.tensor.matmul(pio[:, r0:r1], selsb8[:, 0, :],
                                         uiou_sb[:, 0, r0:r1],
                                         start=False, stop=False)
                        nc.tensor.matmul(pio[:, r0:r1], selsb8[:, 1, :],
                                         uiou_sb[:, 1, r0:r1],
                                         start=False, stop=not with_biou)
                        if with_biou:
                            nc.tensor.matmul(pio[:, r0:r1], ones_sb[:],
                                             bias_sb[:, r0:r1],
                                             start=False, stop=True)
                    nc.scalar.activation(sio_u[:], pio[:, 0:768], SIG,
                                         scale=1.0 / SCALE)
                    # c~ = (sig(2u) - 0.5) * sig(i)  [+ c~_agg]
                    c_t = c_lv[t // 4][:, t % 4, :]
                    nc.vector.scalar_tensor_tensor(
                        c_t, sio_u[:, 2 * H:3 * H], -0.5,
                        sio_u[:, 0:H], op0=ADD, op1=MULT)
                    if d < DEPTH - 1:
                        nc.vector.tensor_add(c_t, c_t, caggs)
                    pend.append((t, sio_u))
                    if len(pend) == 6:
                        flush_pend(4)
                    if t == T - 1:
                        while pend:
                            flush_pend(min(4, len(pend)))

                issue_gather(0)
                issue_gather(1)
                issue_selch(0)
                issue_selch(1)
                if leafch:
                    for g in range(6):
                        issue_hcg(g)
                for k in range(6):
                    stagesAB(k, k - 1)
                for t in range(T):
                    if t % XGG == 0 and t > 0:
                        issue_gather(t // XGG + 1)
                    issue_selch(((t + 4) * m) // SELCH)
                    if leafch:
                        issue_hcg((2 * t + 27) // HCG)
                    stagesAB(t + 6, t + 5)
                    stageC(t)
                    if t >= 1:
                        stageDE(t - 1)
                stageDE(T - 1)
                state[d] = (h_lv, c_lv)

            # ---- final linear on the roots ----
            h0 = state[0][0][0][:, 0, :]
            hrT = transpose256(h0, "hrT")
            plin = p256_pool.tile([128, 128], F32, tag="p256", name="plin")
            pairs = [(lin_sb[:, 0, :], hrT[:, 0:128]),
                     (lin_sb[:, 1, :], hrT[:, 128:256])]
            if with_linb:
                pairs.append((bias_sb[:, 4 * H:4 * H + NCLS], ones_sb[:]))
            matmul_group(plin[0:NCLS, :], pairs)
            out_sb = cpool.tile([128, 128], F32)
            nc.vector.tensor_copy(out_sb[0:NCLS, :], plin[0:NCLS, :])
            nc.sync.dma_start(logits_t[:], out_sb[0:NCLS, :])

    nc.compile()
    return nc


# --------------------------------------------------------------------------
# host wrapper
# --------------------------------------------------------------------------

def prepare(inputs):
    """Returns ((Ls, Ts, ms, flags), in_maps)."""
    x = np.asarray(inputs["x"]).astype(np.int64)
    par = np.asarray(inputs["par"]).astype(np.int64)
    emb = np.asarray(inputs["emb"], dtype=np.float32).copy()
    emb[PAD] = 0.0
    W = np.asarray(inputs["W_iou"], np.float32).copy()
    U = np.asarray(inputs["U_iou"], np.float32).copy()
    Uf = np.asarray(inputs["U_f_w"], np.float32)
    lin = np.asarray(inputs["lin_w"], np.float32)
    b_iou = np.asarray(inputs["b_iou"], np.float32).reshape(-1).copy()
    ufb = np.asarray(inputs["U_f_b"], np.float32).reshape(-1)
    linb = np.asarray(inputs["lin_b"], np.float32).reshape(-1)

    # ---- leaf folding: a leaf's (h, c) depends only on its token, so the
    # whole leaf-level LSTM cell is precomputed per vocab entry on the host.
    # Table rows are [h_leaf | c_leaf/2] (c~ = c/2 is what the device
    # tracks).
    iou_leaf = emb @ W.T + b_iou[None, :]
    il, ol, ul = (iou_leaf[:, 0:H], iou_leaf[:, H:2 * H],
                  iou_leaf[:, 2 * H:3 * H])
    sig = lambda v: 1.0 / (1.0 + np.exp(-v))  # noqa: E731
    c_leaf = sig(il) * np.tanh(ul)
    h_leaf = sig(ol) * np.tanh(c_leaf)
    hc8 = np.concatenate([h_leaf, 0.5 * c_leaf], axis=1).astype(np.float16)
    # leaf h again as a scaled fp8 table for the transposed gather feeding
    # the U_f DoubleRow lhsT (the 16-bit-granularity transpose interleaves
    # fp8 pairs: partition p holds dims (2p, 2p+1))
    ht8 = (h_leaf * SH).astype(ml_dtypes.float8_e4m3)

    # fold the x @ W_iou^T product into the embedding table; pre-scale the
    # u-slice by 2 (the device tracks c~ = c/2 and computes
    # tanh(u) via 2*sigmoid(2u) - 1)
    W[2 * H:3 * H] *= 2.0
    U[2 * H:3 * H] *= 2.0
    b_iou[2 * H:3 * H] *= 2.0
    # U_iou / U_f run as fp8e4m3 DoubleRow matmuls.  fp8's dynamic range
    # bottoms out near these weights' natural ~0.05 magnitude, so the
    # weights carry a x SU scale and the transposed-h operands a x SH
    # scale; the iou/f sigmoids read PSUM with scale 1/(SU*SH).  The
    # x-side preacts (emb8) and biases are pre-scaled to match.
    emb8 = (emb @ W.T * SCALE).astype(np.float16)

    lay = build_layouts(x, par)
    Ls, Ts, ms = lay["L"], lay["T"], lay["m"]

    uiou8T = np.ascontiguousarray(U.T.reshape(2, 128, 3 * H) * SCALE).astype(
        np.float16)
    ufT = np.ascontiguousarray(Uf.T.reshape(2, 128, H) * SU).astype(
        ml_dtypes.float8_e4m3)
    # interleaved-row variant matching the fp8 transposed-gather layout:
    # (p, j) holds U_f^T row 2p+j
    ufT_il = np.ascontiguousarray(Uf.T.reshape(128, 2, H) * SU).astype(
        ml_dtypes.float8_e4m3)
    linT = np.ascontiguousarray(lin.T.reshape(2, 128, NCLS)).astype(
        np.float16)
    biases = np.concatenate([b_iou * SCALE, ufb * SCALE,
                             linb]).astype(np.float16)[None, :]

    flags = dict(with_biou=bool(np.any(b_iou)), with_ufb=bool(np.any(ufb)),
                 with_linb=bool(np.any(linb)))

    in_maps = []
    for c in range(NCORES):
        cl = lay["cores"][c]
        xtokc = np.concatenate([_wrap16(cl["xtok"][d]) for d in range(DEPTH)],
                               axis=1)
        xtokc = np.tile(xtokc, (8, 1))  # replicate across the 8 Q7 cores
        nsel = sum(Ts[d] * ms[d] for d in range(DEPTH - 1))
        rel = np.concatenate(
            [cl["relslot"][d].reshape(-1, 128) for d in range(DEPTH - 1)],
            axis=0)  # [nsel, 128] float16 rel slot per (sel, child-part)
        # one-hot selector matrices, laid out [child_part, sel, parent_slot]
        sel1h = (rel[:, :, None] ==
                 np.arange(128, dtype=np.float32)[None, None, :])
        selm = np.ascontiguousarray(
            sel1h.transpose(1, 0, 2)).astype(np.float16)
        if nsel == 0:
            selm = np.zeros((128, 1, 128), np.float16)
        in_maps.append({
            "emb8": emb8,
            "hc8": hc8,
            "ht8": ht8,
            "uiou8T": uiou8T,
            "ufT": ufT,
            "ufT_il": ufT_il,
            "linT": linT,
            "biases": biases,
            "xtok": np.ascontiguousarray(xtokc).astype(np.int16),
            "selmat": selm,
        })
    return (Ls, Ts, ms, flags), in_maps


_PROGRAM_CACHE = {}


def get_program(Ls, Ts, ms, flags):
    key = (tuple(Ls), tuple(ms), tuple(sorted(flags.items())))
    if key not in _PROGRAM_CACHE:
        _PROGRAM_CACHE[key] = build_program(Ls, Ts, ms, **flags)
    return _PROGRAM_CACHE[key]


def _ensure_device_backend():
    """run_bass_kernel_spmd executes via jax.devices(); make sure those are
    the NeuronCores, not a host-emulation platform."""
    import jax
    try:
        if jax.devices()[0].platform in ("neuron", "axon"):
            return
    except Exception:
        pass
    try:
        jax.config.update("jax_platforms", "neuron")
        jax.clear_backends()
        assert jax.devices()[0].platform in ("neuron", "axon")
    except Exception:
        pass


def kernel(**inputs):
    _ensure_device_backend()
    (Ls, Ts, ms, flags), in_maps = prepare(inputs)
    nc = get_program(Ls, Ts, ms, flags)
    res = run_bass_kernel_spmd(nc, in_maps, core_ids=list(range(NCORES)))
    logits = np.zeros((B, NCLS), np.float32)
    for c in range(NCORES):
        lt = res.results[c]["logits_t"]  # [104, 128]
        logits[c * TPC:(c + 1) * TPC] = np.asarray(lt)[:, 0:TPC].T
    return logits



# revision 61
# speedup vs baseline: 1.0099x; 1.0099x over previous
"""Trainium2 Bass kernel for the DGL ChildSum-TreeLSTM problem.

Strategy (per spec sharding hint): 32 independent trees -> 4 trees per
NeuronCore, weights replicated, logits gathered on host.

Per-core computation is restructured around a host-computed "slot layout":
every level's nodes are permuted so that the children of each 128-parent
tile sit in a contiguous, chunk-aligned window of the next level's node
order.  The irregular-fan-in segment_sum then becomes a handful of
128x128 one-hot selector matmuls accumulating in PSUM, and all LSTM state
stays resident in SBUF in fp16 (no DRAM traffic for states at all).

Key optimizations (in order of impact):
 - **Leaf folding**: a leaf's (h, c) is a pure function of its token, so
   the whole leaf level (half of all nodes) is precomputed on the host
   into a [VOCAB, 512] fp16 table [h | c~] and row-gathered on device --
   no leaf-level compute at all.
 - The x @ W_iou^T product for non-leaf nodes is folded into the
   embedding table on the host (emb_iou, fp16, pre-scaled by SCALE), so
   their only data-dependent DMA is a row gather of iou preactivations.
 - The u-slice of W/U/b is pre-scaled by 2 on the host and c~ = c/2 is
   tracked on device, which turns tanh(u) into an affine image of
   sigmoid(2u): ONE sigmoid per node tile covers i, o and u, and
   tanh(c) = tanh(2*c~) is an activation-scale.  c~ stays exact through
   the linear child-sum recursion.
 - U_f runs as an fp8e4m3 DoubleRow matmul (256-contraction in one
   instruction at 0.5 cycles/row); weights carry x SU and the
   transposed-h operand x SH, undone by the f-sigmoid's 1/SCALE.  The
   f-gate path tolerates fp8 (~5e-4 rel err); U_iou and the x path do
   NOT (tested ~3e-2) and stay fp16.
 - Engine assignment follows per-PHASE occupancy: during the leaf /
   level-10 startup Pool is busy with SWDGE descriptor generation, so
   leaf fc-muls and fp8 conversions run on the otherwise-idle DVE;
   for levels 8-9 DVE is the bottleneck, so fc/h-muls go to GPSIMD.
 - tanh(c) is batched over quads of node tiles; selector matrices are
   fetched in 12-window chunks, small-level selectors and emb_iou rows
   in one hoisted prefetch emitted mid-pipeline (level 9) to keep the
   startup DMA window clear for the leaf-table gathers.
"""

import numpy as np
import ml_dtypes

import concourse.bacc as bacc
import concourse.bass as bass  # noqa: F401
import concourse.mybir as mybir
import concourse.tile as tile
from concourse.bass_utils import run_bass_kernel_spmd
from concourse.masks import make_identity

# ---- static problem structure (from the reference nn.Module) ----
B = 32
DEPTH = 12
LEVELS = [2 ** d for d in range(DEPTH)]
_STARTS = [0]
for _l in LEVELS[:-1]:
    _STARTS.append(_STARTS[-1] + _l)
NPG = sum(LEVELS)              # 4095 nodes per tree
N = B * NPG
VOCAB = 20000
PAD = VOCAB - 1
E = 256
H = 256
NCLS = 104
NCORES = 8
TPC = B // NCORES              # trees per core
F8 = mybir.dt.float8e4
F16 = mybir.dt.float16
F32 = mybir.dt.float32
I16 = mybir.dt.int16
GATHER_GROUP = 6               # node tiles per xsm prefetch gather
XGG = 6                        # node tiles per emb_iou gather group
USE_HT_GATHER = False           # fetch leaf h pre-transposed via dma_gather
SU = 16.0                      # fp8 U_iou / U_f weight scale
SH = 8.0                       # fp8 transposed-h operand scale
SCALE = SU * SH                # combined PSUM preact scale
SIG = mybir.ActivationFunctionType.Sigmoid
TANH = mybir.ActivationFunctionType.Tanh
COPYF = mybir.ActivationFunctionType.Copy
DR = mybir.MatmulPerfMode.DoubleRow
ADD = mybir.AluOpType.add
MULT = mybir.AluOpType.mult


# --------------------------------------------------------------------------
# host-side slot layout
# --------------------------------------------------------------------------

def _core_layout(x, par, core):
    """Per-core slot assignment. Returns per level: node-id per slot and the
    window size m (see module docstring)."""
    trees = range(core * TPC, (core + 1) * TPC)
    slot_nodes = [np.array([g * NPG for g in trees], dtype=np.int64)]
    levels = []
    for d in range(DEPTH):
        nodes = slot_nodes[d]
        Lpad = ((len(nodes) + 127) // 128) * 128
        if Lpad > len(nodes):
            nodes = np.concatenate([nodes, np.full(Lpad - len(nodes), -1,
                                                   np.int64)])
        slot_nodes[d] = nodes
        T = Lpad // 128
        lv = {"nodes": nodes, "T": T, "L": Lpad, "m": 0}
        levels.append(lv)
        if d == DEPTH - 1:
            continue
        # children of every real node in this level, grouped by parent
        ch_by_parent = {}
        for g in trees:
            lo = g * NPG + _STARTS[d + 1]
            hi = lo + LEVELS[d + 1]
            p = np.asarray(par[lo:hi])
            order = np.argsort(p, kind="stable")
            ids = np.arange(lo, hi, dtype=np.int64)[order]
            ps = p[order]
            uniq, start_idx = np.unique(ps, return_index=True)
            bounds = list(start_idx) + [len(ps)]
            for i, u in enumerate(uniq):
                ch_by_parent[int(u)] = ids[bounds[i]:bounds[i + 1]]
        cursor = 0
        child_slots = []
        m = 2
        for t in range(T):
            start = max(256 * t, cursor)
            child_slots.extend([-1] * (start - cursor))
            kids = []
            for s in range(128):
                node = nodes[t * 128 + s]
                if node >= 0:
                    kids.extend(ch_by_parent.get(int(node), ()))
            child_slots.extend(kids)
            cursor = start + len(kids)
            if cursor > 256 * t:
                m = max(m, -(-(cursor - 256 * t) // 128))
        lv["m"] = m
        slot_nodes.append(np.array(child_slots, dtype=np.int64))
    return levels


def build_layouts(x, par):
    x = np.asarray(x)
    par = np.asarray(par)
    cores = [_core_layout(x, par, c) for c in range(NCORES)]
    Ls = [max(cores[c][d]["L"] for c in range(NCORES)) for d in range(DEPTH)]
    Ts = [L // 128 for L in Ls]
    ms = [max(cores[c][d]["m"] for c in range(NCORES)) for d in range(DEPTH)]
    out = {"L": Ls, "T": Ts, "m": ms, "cores": []}
    for c in range(NCORES):
        lvs = cores[c]
        node_slots = []
        xtok = []
        for d in range(DEPTH):
            nodes = lvs[d]["nodes"]
            if len(nodes) < Ls[d]:
                nodes = np.concatenate(
                    [nodes, np.full(Ls[d] - len(nodes), -1, np.int64)])
            node_slots.append(nodes)
            tok = np.where(nodes >= 0, np.asarray(x)[np.maximum(nodes, 0)],
                           PAD)
            xtok.append(tok.astype(np.int16))
        relslot = []
        for d in range(DEPTH - 1):
            T, m = Ts[d], ms[d]
            child_nodes = node_slots[d + 1]
            rel = np.full((T, m, 128), -1.0, np.float16)
            pslot_of = {int(n): i for i, n in enumerate(node_slots[d])
                        if n >= 0}
            nch = Ls[d + 1] // 128
            for j in range(nch):
                for p in range(128):
                    node = child_nodes[j * 128 + p]
                    if node < 0:
                        continue
                    ps = pslot_of[int(par[node])]
                    t, sl = ps // 128, ps % 128
                    w = j - 2 * t
                    assert 0 <= w < m
                    rel[t, w, p] = sl
            relslot.append(rel)
        out["cores"].append({"xtok": xtok, "relslot": relslot,
                             "nodes": node_slots})
    return out


# --------------------------------------------------------------------------
# device program
# --------------------------------------------------------------------------

def _wrap16(tok):
    """int16 [L] -> the dma_gather 16-partition wrapped layout [16, L//16]."""
    return tok.reshape(-1, 16).T.copy()


def _ilhs(gt_ap, ni, off):
    """DoubleRow lhsT view [128, 2, 128] into a [128, 2, ni] fp8
    transpose-gather tile.  The gather writes, per partition p, an
    idx-major byte stream: byte q holds dim (2p + q%2) of idx q//2, so
    nodes off..off+127 occupy the 256 contiguous bytes at 2*off; the DR
    k-tile axis is the byte parity."""
    jj, oo = divmod(2 * off, ni)
    return gt_ap[:, jj, oo:oo + 256].rearrange("p (m b) -> p b m", b=2)


def build_program(Ls, Ts, ms, with_biou, with_ufb, with_linb):
    nch_of = [Ls[d] // 128 for d in range(DEPTH)]
    idx_cols = [Ls[d] // 16 for d in range(DEPTH)]
    idx_off = np.concatenate([[0], np.cumsum(idx_cols)]).astype(int)
    sel_cnt = [Ts[d] * ms[d] for d in range(DEPTH - 1)]
    sel_off = np.concatenate([[0], np.cumsum(sel_cnt)]).astype(int)
    nsel = int(sel_off[-1])

    nc = bacc.Bacc("TRN2", debug=False, num_devices=NCORES)

    emb8 = nc.dram_tensor("emb8", [VOCAB, 3 * H], F16,
                          kind="ExternalInput").ap()
    hc8 = nc.dram_tensor("hc8", [VOCAB, 2 * H], F16,
                         kind="ExternalInput").ap()
    ht8 = nc.dram_tensor("ht8", [VOCAB, H], F8,
                         kind="ExternalInput").ap()
    ufT_il = nc.dram_tensor("ufT_il", [128, 2, H], F8,
                            kind="ExternalInput").ap()
    uiou8T = nc.dram_tensor("uiou8T", [2, 128, 3 * H], F16,
                            kind="ExternalInput").ap()
    ufT = nc.dram_tensor("ufT", [2, 128, H], F8,
                         kind="ExternalInput").ap()
    linT = nc.dram_tensor("linT", [2, 128, NCLS], F16,
                          kind="ExternalInput").ap()
    biases = nc.dram_tensor("biases", [1, 3 * H + H + NCLS], F16,
                            kind="ExternalInput").ap()
    xtok = nc.dram_tensor("xtok", [128, int(idx_off[-1])], I16,
                          kind="ExternalInput").ap()
    selmat = nc.dram_tensor("selmat", [128, max(nsel, 1), 128], F16,
                            kind="ExternalInput").ap()
    logits_t = nc.dram_tensor("logits_t", [NCLS, 128], F32,
                              kind="ExternalOutput").ap()

    with tile.TileContext(nc) as tc:
        with (
            tc.tile_pool(name="const", bufs=1) as cpool,
            tc.tile_pool(name="state", bufs=1) as spool,
            tc.tile_pool(name="hcg", bufs=7) as hcpool,
            tc.tile_pool(name="hT", bufs=3) as htpool,
            tc.tile_pool(name="xg", bufs=2) as xpool,
            tc.tile_pool(name="sio", bufs=7) as siopool,
            tc.tile_pool(name="sel", bufs=4) as selpool,
            tc.tile_pool(name="work", bufs=6) as wpool,
            tc.tile_pool(name="chunk", bufs=4) as chpool,
            tc.tile_pool(name="psel", bufs=2, space="PSUM") as psel_pool,
            tc.tile_pool(name="p256", bufs=2, space="PSUM") as p256_pool,
            tc.tile_pool(name="piou", bufs=2, space="PSUM") as piou_pool,
        ):
            # ---- resident constants ----
            uiou_sb = cpool.tile([128, 2, 3 * H], F16)
            uf_sb = cpool.tile([128, 2, H], F8)
            uf_il_sb = cpool.tile([128, 2, H], F8)
            nc.sync.dma_start(uf_il_sb[:], ufT_il[:])
            lin_sb = cpool.tile([128, 2, NCLS], F16)
            bias_sb = cpool.tile([1, 3 * H + H + NCLS], F16)
            ones_sb = cpool.tile([1, 128], F16)
            idx_sb = cpool.tile([128, int(idx_off[-1])], I16)
            nc.sync.dma_start(idx_sb[:], xtok[:])
            for k in range(2):
                nc.sync.dma_start(uiou_sb[:, k, :], uiou8T[k])
                nc.sync.dma_start(uf_sb[:, k, :], ufT[k])
                nc.sync.dma_start(lin_sb[:, k, :], linT[k])
            nc.sync.dma_start(bias_sb[:], biases[:])
            nc.vector.memset(ones_sb[:], 1.0)
            ident = cpool.tile([128, 128], F16)
            make_identity(nc, ident[:])

            # ---- hoisted prefetch for the small top levels ----
            SMALL_IDX = 1536   # first 1536 slots = levels 0..7 here
            small_set = set(d for d in range(DEPTH)
                            if int(idx_off[d + 1]) * 16 <= SMALL_IDX)
            n_sm_g = (min(SMALL_IDX, int(idx_off[-1]) * 16)
                      + 767) // 768
            xsm = []
            nsel_sm = int(sel_off[max(small_set) + 1]) if small_set else 0
            s3sm = (cpool.tile([128, nsel_sm, 128], F16, name="s3sm")
                    if nsel_sm else None)

            def small_prefetch():
                """Emitted after the leaf level so these bulk transfers do
                not contend with the first leaf gathers on the DMA engines."""
                for g in range(n_sm_g):
                    xs = cpool.tile([128, GATHER_GROUP, 3 * H], F16,
                                    name=f"xsm{g}")
                    nc.gpsimd.dma_gather(
                        xs[:, :, :], emb8[:],
                        idx_sb[:, g * 48:(g + 1) * 48],
                        768, 768, 3 * H, transpose=False)
                    xsm.append(xs)
                if nsel_sm:
                    nc.sync.dma_start(s3sm[:], selmat[:, 0:nsel_sm, :])

            state = {}   # level -> (h quads, c quads), each [128, 4, H]
            SELCH = 12   # selectors per fetch chunk

            def transpose256(src_ap, tag):
                """[128,256] fp16 -> [128,256] fp16 holding the two
                transposed 128x128 halves side by side."""
                pt = p256_pool.tile([128, 256], F16, tag="p256", name="pt")
                nc.tensor.transpose(pt[:, 0:128], src_ap[:, 0:128], ident[:])
                nc.tensor.transpose(pt[:, 128:256], src_ap[:, 128:256],
                                    ident[:])
                dst = wpool.tile([128, 256], F16, tag=tag, bufs=1,
                                 name=tag)
                nc.vector.tensor_copy(dst[:], pt[:])
                return dst

            def matmul_group(out_ap, pairs):
                """Emit an accumulation group into one psum zero-region."""
                for i, (lhsT, rhs) in enumerate(pairs):
                    nc.tensor.matmul(out_ap, lhsT, rhs, start=(i == 0),
                                     stop=(i == len(pairs) - 1))

            # state slots per parity class: enough quad-tiles for the
            # largest level of that parity (the leaf level has no state
            # tiles: its (h, c~) arrives pre-computed via dma_gather)
            state_bufs = [0, 0]
            for d in range(DEPTH - 1):
                state_bufs[d % 2] = max(state_bufs[d % 2],
                                        (Ts[d] + 3) // 4)

            HCG = 6              # leaf hc row-gather group (tiles)
            HTCH = 2048          # idx per transposed-h gather chunk

            for d in range(DEPTH - 1, -1, -1):
                L, T, m = Ls[d], Ts[d], ms[d]
                if d == DEPTH - 1:
                    # ---- leaf level: (h, c~) gathered from the host-folded
                    # per-token table; h additionally gathered transposed
                    # (feature-major) for the U_f lhsT of the level above.
                    lbase = int(idx_off[d])
                    hcg, hTc = {}, {}

                    def issue_hcg(g, hcg=hcg, lbase=lbase, T=T):
                        if g < 0 or g >= -(-T // HCG) or g in hcg:
                            return
                        gt = min(HCG, T - g * HCG)
                        ht_ = hcpool.tile([128, gt, 2 * H], F16,
                                          tag="hcg", name="hcg")
                        nc.gpsimd.dma_gather(
                            ht_[:, :, :], hc8[:],
                            idx_sb[:, lbase + g * HCG * 8:
                                   lbase + g * HCG * 8 + gt * 8],
                            gt * 128, gt * 128, 2 * H, transpose=False)
                        hcg[g] = ht_

                    def issue_hT(k, hTc=hTc, lbase=lbase, T=T):
                        if k < 0 or k >= -(-(T * 128) // HTCH) or k in hTc:
                            return
                        ni = min(HTCH, T * 128 - k * HTCH)
                        tt_ = htpool.tile([128, 2, ni], F8, tag="hT",
                                          name="hT")
                        nc.gpsimd.dma_gather(
                            tt_[:, :, :], ht8[:],
                            idx_sb[:, lbase + k * (HTCH // 16):
                                   lbase + (k * HTCH + ni) // 16],
                            ni, ni, H, transpose=True)
                        hTc[k] = tt_

                    issue_hcg(0)
                    if USE_HT_GATHER:
                        issue_hT(0)
                    issue_hcg(1)
                    if not USE_HT_GATHER:
                        def issue_hT(k):  # noqa: F811
                            pass
                    state[d] = ("leaf", hcg, hTc, issue_hcg, issue_hT)
                    continue
                if d == 9:
                    small_prefetch()
                nquad = (T + 3) // 4
                h_lv = [spool.tile([128, 4, H], F16, tag=f"h{d % 2}",
                                   bufs=state_bufs[d % 2],
                                   name=f"h{d}_{q}") for q in range(nquad)]
                c_lv = [spool.tile([128, 4, H], F16, tag=f"c{d % 2}",
                                   bufs=state_bufs[d % 2],
                                   name=f"c{d}_{q}") for q in range(nquad)]
                sel_ch = {}
                nselch = (-(-(T * m) // SELCH)
                          if (d < DEPTH - 1 and d not in small_set) else 0)

                def issue_selch(ci, d=d, sel_ch=sel_ch, nselch=nselch):
                    if ci >= nselch or ci in sel_ch:
                        return
                    cnt = min(SELCH, T * m - ci * SELCH)
                    st = selpool.tile([128, cnt, 128], F16, tag="sel",
                                      name="selch")
                    base = int(sel_off[d]) + ci * SELCH
                    nc.sync.dma_start(st[:], selmat[:, base:base + cnt, :])
                    sel_ch[ci] = st
                if d < DEPTH - 1:
                    child = state[d + 1]
                    leafch = isinstance(child, tuple) and child[0] == "leaf"
                    nch = nch_of[d + 1]
                    if leafch:
                        _, hcg, hTc, issue_hcg, issue_hT = child

                        def h_half(j, k, hcg=hcg):
                            return hcg[j // HCG][:, j % HCG,
                                                 k * 128:(k + 1) * 128]

                        def h_full(j, hcg=hcg):
                            return hcg[j // HCG][:, j % HCG, 0:H]

                        def c_rows(p, n, hcg=hcg):
                            g, r = (2 * p) // HCG, (2 * p) % HCG
                            return hcg[g][:, r:r + n, H:2 * H]

                        def hT_lhs(j, hTc=hTc):
                            ck, off = divmod(j * 128, HTCH)
                            ni = min(HTCH, Ls[DEPTH - 1] - ck * HTCH)
                            return _ilhs(hTc[ck][:, :, :], ni, off)
                    else:
                        h_ch, c_ch = child

                        def h_half(j, k, h_ch=h_ch):
                            return h_ch[j // 4][:, j % 4,
                                               k * 128:(k + 1) * 128]

                        def h_full(j, h_ch=h_ch):
                            return h_ch[j // 4][:, j % 4, :]

                        def c_rows(p, n, c_ch=c_ch):
                            q, r = (2 * p) // 4, (2 * p) % 4
                            return c_ch[q][:, r:r + n, :]

                        hT_lhs = None
                    chunk_pt = {}   # pair -> (hjT2, js_p) after stage A
                    chunk_fc = {}   # chunk -> fc AP after stage B

                    def stageA(p):
                        """PE transposes of the pair's h + psum->sbuf copy.
                        For leaf children h^T arrives via the transposed
                        gather, so the stage is dependency-tracking only."""
                        js_p = [j for j in (2 * p, 2 * p + 1) if j < nch]
                        if not js_p:
                            return
                        if leafch and USE_HT_GATHER:
                            chunk_pt[p] = (None, js_p)
                            return
                        w = len(js_p) * H
                        pt2 = p256_pool.tile([128, w], F16, tag="p256",
                                             name="pt2")
                        for ji, j in enumerate(js_p):
                            for k in range(2):
                                nc.tensor.transpose(
                                    pt2[:, ji * H + k * 128:
                                        ji * H + (k + 1) * 128],
                                    h_half(j, k), ident[:])
                        hjT2 = chpool.tile([128, 2 * len(js_p), 128], F8,
                                           tag="hjT", bufs=3, name="hjT2")
                        nc.vector.tensor_scalar_mul(hjT2[:], pt2[:], SH)
                        chunk_pt[p] = (hjT2, js_p)

                    def stageB(p):
                        """U_f fp8 DoubleRow matmul + sigmoid + f*c."""
                        if p not in chunk_pt:
                            return
                        hjT2, js_p = chunk_pt.pop(p)
                        w = len(js_p) * H
                        pf2 = p256_pool.tile([128, w], F32, tag="p256",
                                             name="pf2")
                        for ji, j in enumerate(js_p):
                            if hjT2 is None:
                                lhsT, rhs = hT_lhs(j), uf_il_sb[:, :, :]
                            else:
                                lhsT = hjT2[:, 2 * ji:2 * ji + 2, :]
                                rhs = uf_sb[:, :, :]
                            nc.tensor.matmul(
                                pf2[:, ji * H:(ji + 1) * H], lhsT, rhs,
                                perf_mode=DR, start=True,
                                stop=not with_ufb)
                            if with_ufb:
                                nc.tensor.matmul(
                                    pf2[:, ji * H:(ji + 1) * H], ones_sb[:],
                                    bias_sb[:, 3 * H:4 * H],
                                    start=False, stop=True)
                        fj2 = chpool.tile([128, w], F16, tag="fj",
                                          bufs=2, name="fj2")
                        nc.scalar.activation(fj2[:], pf2[:], SIG,
                                             scale=1.0 / SCALE)
                        fc2 = chpool.tile([128, w], F16, tag="fcj",
                                          bufs=6, name="fc2")
                        fc_eng = (nc.gpsimd if d in (8, 9)
                                  else nc.vector)
                        fc_eng.tensor_mul(fc2[:], fj2[:],
                                          c_rows(p, len(js_p)))
                        for ji, j in enumerate(js_p):
                            chunk_fc[j] = fc2[:, ji * H:(ji + 1) * H]

                # xio for this level: hoisted prefetch for small levels,
                # grouped dma_gathers (issued inside the t-loop) otherwise
                xg = {}

                def issue_gather(g, d=d, xg=xg):
                    if d in small_set or g * XGG >= T:
                        return
                    gt = min(XGG, T - g * XGG)
                    gi = gt * 128
                    xt = xpool.tile([128, gt, 3 * H], F16, tag="xt",
                                    name="xt")
                    nc.gpsimd.dma_gather(
                        xt[:, :, :], emb8[:],
                        idx_sb[:, int(idx_off[d]) + g * XGG * 8:
                               int(idx_off[d]) + g * XGG * 8
                               + gt * 8],
                        gi, gi, 3 * H, transpose=False)
                    xg[g] = xt

                if d in small_set:
                    eb = int(idx_off[d]) * 16

                    def xslice(t, eb=eb):
                        g, off = divmod(eb + t * 128, 768)
                        return xsm[g][:, off // 128, :]
                else:
                    def xslice(t, xg=xg):
                        return xg[t // XGG][:, t % XGG, :]

                def js_of(t):
                    return [2 * t + w for w in range(m) if 2 * t + w < nch]

                def pairs_of(t):
                    return sorted({j // 2 for j in js_of(t)})

                def Sw_of(t, wi):
                    if d in small_set:
                        return s3sm[:, int(sel_off[d]) + t * m + wi, :]
                    si = t * m + wi
                    return sel_ch[si // SELCH][:, si % SELCH, :]

                selsb_q = {}
                doneA, doneB = set(), set()

                def stagesAB(tA, tB):
                    if 0 <= tA < T:
                        for p in pairs_of(tA):
                            if p not in doneA:
                                stageA(p)
                                doneA.add(p)
                    if 0 <= tB < T:
                        for p in pairs_of(tB):
                            if p not in doneB:
                                stageB(p)
                                doneB.add(p)

                def stageC(t):
                    psel = psel_pool.tile([128, 2 * H], F32, tag="psel",
                                          name="psel")
                    js = js_of(t)
                    if js:
                        nmm = 3 * len(js)
                        k = 0
                        for wi, j in enumerate(js):
                            Sw = Sw_of(t, wi)
                            nc.tensor.matmul(
                                psel[:, 0:128], h_half(j, 0), Sw,
                                start=(k == 0), stop=(k == nmm - 1))
                            k += 1
                            nc.tensor.matmul(
                                psel[:, 128:256], h_half(j, 1),
                                Sw, start=(k == 0), stop=(k == nmm - 1))
                            k += 1
                            nc.tensor.matmul(
                                psel[:, 2 * 128:2 * 128 + H], Sw,
                                chunk_fc[j][:],
                                start=(k == 0), stop=(k == nmm - 1))
                            k += 1
                    else:
                        raise AssertionError("empty selector window")
                    # h~^T halves + c_agg -> fp16 SBUF (psel retires fast;
                    # the later c~ add then runs in the DVE 2x mode)
                    selsb8 = wpool.tile([128, 2, 128], F16, tag="s8",
                                        bufs=3, name="s8")
                    caggsb = wpool.tile([128, 256], F16, tag="cagg",
                                        bufs=3, name="cagg")
                    nc.vector.tensor_copy(selsb8[:], psel[:, 0:256])
                    nc.vector.tensor_copy(caggsb[:], psel[:, 256:512])
                    selsb_q[t] = (selsb8, caggsb[:])

                pend = []   # (t, sio_u) awaiting the quad tanh(c) + h mul

                def flush_pend(k):
                    grp = pend[:k]
                    del pend[:k]
                    t0 = grp[0][0]
                    npr = len(grp)
                    tcn = wpool.tile([128, npr, H], F16, tag="tcn",
                                     bufs=2, name="tcn")
                    nc.scalar.activation(tcn[:],
                                         c_lv[t0 // 4][:, 0:npr, :],
                                         TANH, scale=2.0)
                    mul_eng = nc.gpsimd if d in (8, 9) else nc.vector
                    for i, (tt, sio_t) in enumerate(grp):
                        mul_eng.tensor_mul(h_lv[tt // 4][:, tt % 4, :],
                                           sio_t[:, 2 * H:3 * H],
                                           tcn[:, i, :])

                def stageDE(t):
                    sio_u = siopool.tile([128, 3 * H], F16, tag="sio",
                                         name="sio")
                    selsb8, caggs = selsb_q.pop(t)
                    pio = piou_pool.tile([128, 768], F32, tag="pio",
                                         name="pio")
                    for r0, r1 in ((0, 512), (512, 768)):
                        nc.tensor.matmul(pio[:, r0:r1], ident[:],
                                         xslice(t)[:, r0:r1],
                                         start=True, stop=False)
                        nc.tensor.matmul(pio[:, r0:r1], selsb8[:, 0, :],
                                         uiou_sb[:, 0, r0:r1],
                                         start=False, stop=False)
                        nc.tensor.matmul(pio[:, r0:r1], selsb8[:, 1, :],
                                         uiou_sb[:, 1, r0:r1],
                                         start=False, stop=not with_biou)
                        if with_biou:
                            nc.tensor.matmul(pio[:, r0:r1], ones_sb[:],
                                             bias_sb[:, r0:r1],
                                             start=False, stop=True)
                    if T <= 4:
                        # tail levels: the (i, u) pair gates the serial c~
                        # chain; sigmoid it first, o afterwards
                        nc.scalar.activation(sio_u[:, 0:512],
                                             pio[:, 0:512], SIG,
                                             scale=1.0 / SCALE)
                        nc.scalar.activation(sio_u[:, 512:768],
                                             pio[:, 512:768], SIG,
                                             scale=1.0 / SCALE)
                    else:
                        nc.scalar.activation(sio_u[:], pio[:, 0:768], SIG,
                                             scale=1.0 / SCALE)
                    # c~ = (sig(2u) - 0.5) * sig(i)  [+ c~_agg]
                    c_t = c_lv[t // 4][:, t % 4, :]
                    nc.vector.scalar_tensor_tensor(
                        c_t, sio_u[:, H:2 * H], -0.5,
                        sio_u[:, 0:H], op0=ADD, op1=MULT)
                    if d < DEPTH - 1:
                        nc.vector.tensor_add(c_t, c_t, caggs)
                    pend.append((t, sio_u))
                    if len(pend) == 6:
                        flush_pend(4)
                    if t == T - 1:
                        while pend:
                            flush_pend(min(4, len(pend)))

                issue_gather(0)
                issue_gather(1)
                issue_selch(0)
                issue_selch(1)
                if leafch:
                    for g in range(6):
                        issue_hcg(g)
                for k in range(6):
                    stagesAB(k, k - 1)
                for t in range(T):
                    if t % XGG == 0 and t > 0:
                        issue_gather(t // XGG + 1)
                    issue_selch(((t + 4) * m) // SELCH)
                    if leafch:
                        issue_hcg((2 * t + 27) // HCG)
                    stagesAB(t + 6, t + 5)
                    stageC(t)
                    if t >= 1:
                        stageDE(t - 1)
                stageDE(T - 1)
                state[d] = (h_lv, c_lv)

            # ---- final linear on the roots ----
            h0 = state[0][0][0][:, 0, :]
            hrT = transpose256(h0, "hrT")
            plin = p256_pool.tile([128, 128], F32, tag="p256", name="plin")
            pairs = [(lin_sb[:, 0, :], hrT[:, 0:128]),
                     (lin_sb[:, 1, :], hrT[:, 128:256])]
            if with_linb:
                pairs.append((bias_sb[:, 4 * H:4 * H + NCLS], ones_sb[:]))
            matmul_group(plin[0:NCLS, :], pairs)
            out_sb = cpool.tile([128, 128], F32)
            nc.vector.tensor_copy(out_sb[0:NCLS, :], plin[0:NCLS, :])
            nc.sync.dma_start(logits_t[:], out_sb[0:NCLS, :])

    nc.compile()
    return nc


# --------------------------------------------------------------------------
# host wrapper
# --------------------------------------------------------------------------

def prepare(inputs):
    """Returns ((Ls, Ts, ms, flags), in_maps)."""
    x = np.asarray(inputs["x"]).astype(np.int64)
    par = np.asarray(inputs["par"]).astype(np.int64)
    emb = np.asarray(inputs["emb"], dtype=np.float32).copy()
    emb[PAD] = 0.0
    W = np.asarray(inputs["W_iou"], np.float32).copy()
    U = np.asarray(inputs["U_iou"], np.float32).copy()
    Uf = np.asarray(inputs["U_f_w"], np.float32)
    lin = np.asarray(inputs["lin_w"], np.float32)
    b_iou = np.asarray(inputs["b_iou"], np.float32).reshape(-1).copy()
    ufb = np.asarray(inputs["U_f_b"], np.float32).reshape(-1)
    linb = np.asarray(inputs["lin_b"], np.float32).reshape(-1)

    # ---- leaf folding: a leaf's (h, c) depends only on its token, so the
    # whole leaf-level LSTM cell is precomputed per vocab entry on the host.
    # Table rows are [h_leaf | c_leaf/2] (c~ = c/2 is what the device
    # tracks).
    iou_leaf = emb @ W.T + b_iou[None, :]
    il, ol, ul = (iou_leaf[:, 0:H], iou_leaf[:, H:2 * H],
                  iou_leaf[:, 2 * H:3 * H])
    sig = lambda v: 1.0 / (1.0 + np.exp(-v))  # noqa: E731
    c_leaf = sig(il) * np.tanh(ul)
    h_leaf = sig(ol) * np.tanh(c_leaf)
    hc8 = np.concatenate([h_leaf, 0.5 * c_leaf], axis=1).astype(np.float16)
    # leaf h again as a scaled fp8 table for the transposed gather feeding
    # the U_f DoubleRow lhsT (the 16-bit-granularity transpose interleaves
    # fp8 pairs: partition p holds dims (2p, 2p+1))
    ht8 = (h_leaf * SH).astype(ml_dtypes.float8_e4m3)

    # fold the x @ W_iou^T product into the embedding table; pre-scale the
    # u-slice by 2 (the device tracks c~ = c/2 and computes
    # tanh(u) via 2*sigmoid(2u) - 1)
    W[2 * H:3 * H] *= 2.0
    U[2 * H:3 * H] *= 2.0
    b_iou[2 * H:3 * H] *= 2.0
    # U_iou / U_f run as fp8e4m3 DoubleRow matmuls.  fp8's dynamic range
    # bottoms out near these weights' natural ~0.05 magnitude, so the
    # weights carry a x SU scale and the transposed-h operands a x SH
    # scale; the iou/f sigmoids read PSUM with scale 1/(SU*SH).  The
    # x-side preacts (emb8) and biases are pre-scaled to match.
    # column order [i | u | o]: the u-slice rides next to i so the tail
    # levels can sigmoid the chain-critical (i, u) pair first
    iuo = np.concatenate([np.arange(0, H), np.arange(2 * H, 3 * H),
                          np.arange(H, 2 * H)])
    emb8 = (emb @ W.T * SCALE)[:, iuo].astype(np.float16)

    lay = build_layouts(x, par)
    Ls, Ts, ms = lay["L"], lay["T"], lay["m"]

    uiou8T = np.ascontiguousarray(
        (U.T * SCALE)[:, iuo].reshape(2, 128, 3 * H)).astype(np.float16)
    ufT = np.ascontiguousarray(Uf.T.reshape(2, 128, H) * SU).astype(
        ml_dtypes.float8_e4m3)
    # interleaved-row variant matching the fp8 transposed-gather layout:
    # (p, j) holds U_f^T row 2p+j
    ufT_il = np.ascontiguousarray(Uf.T.reshape(128, 2, H) * SU).astype(
        ml_dtypes.float8_e4m3)
    linT = np.ascontiguousarray(lin.T.reshape(2, 128, NCLS)).astype(
        np.float16)
    biases = np.concatenate([(b_iou * SCALE)[iuo], ufb * SCALE,
                             linb]).astype(np.float16)[None, :]

    flags = dict(with_biou=bool(np.any(b_iou)), with_ufb=bool(np.any(ufb)),
                 with_linb=bool(np.any(linb)))

    in_maps = []
    for c in range(NCORES):
        cl = lay["cores"][c]
        xtokc = np.concatenate([_wrap16(cl["xtok"][d]) for d in range(DEPTH)],
                               axis=1)
        xtokc = np.tile(xtokc, (8, 1))  # replicate across the 8 Q7 cores
        nsel = sum(Ts[d] * ms[d] for d in range(DEPTH - 1))
        rel = np.concatenate(
            [cl["relslot"][d].reshape(-1, 128) for d in range(DEPTH - 1)],
            axis=0)  # [nsel, 128] float16 rel slot per (sel, child-part)
        # one-hot selector matrices, laid out [child_part, sel, parent_slot]
        sel1h = (rel[:, :, None] ==
                 np.arange(128, dtype=np.float32)[None, None, :])
        selm = np.ascontiguousarray(
            sel1h.transpose(1, 0, 2)).astype(np.float16)
        if nsel == 0:
            selm = np.zeros((128, 1, 128), np.float16)
        in_maps.append({
            "emb8": emb8,
            "hc8": hc8,
            "ht8": ht8,
            "uiou8T": uiou8T,
            "ufT": ufT,
            "ufT_il": ufT_il,
            "linT": linT,
            "biases": biases,
            "xtok": np.ascontiguousarray(xtokc).astype(np.int16),
            "selmat": selm,
        })
    return (Ls, Ts, ms, flags), in_maps


_PROGRAM_CACHE = {}


def get_program(Ls, Ts, ms, flags):
    key = (tuple(Ls), tuple(ms), tuple(sorted(flags.items())))
    if key not in _PROGRAM_CACHE:
        _PROGRAM_CACHE[key] = build_program(Ls, Ts, ms, **flags)
    return _PROGRAM_CACHE[key]


def _ensure_device_backend():
    """run_bass_kernel_spmd executes via jax.devices(); make sure those are
    the NeuronCores, not a host-emulation platform."""
    import jax
    try:
        if jax.devices()[0].platform in ("neuron", "axon"):
            return
    except Exception:
        pass
    try:
        jax.config.update("jax_platforms", "neuron")
        jax.clear_backends()
        assert jax.devices()[0].platform in ("neuron", "axon")
    except Exception:
        pass


def kernel(**inputs):
    _ensure_device_backend()
    (Ls, Ts, ms, flags), in_maps = prepare(inputs)
    nc = get_program(Ls, Ts, ms, flags)
    res = run_bass_kernel_spmd(nc, in_maps, core_ids=list(range(NCORES)))
    logits = np.zeros((B, NCLS), np.float32)
    for c in range(NCORES):
        lt = res.results[c]["logits_t"]  # [104, 128]
        logits[c * TPC:(c + 1) * TPC] = np.asarray(lt)[:, 0:TPC].T
    return logits



# revision 66
# speedup vs baseline: 1.0179x; 1.0079x over previous
"""Trainium2 Bass kernel for the DGL ChildSum-TreeLSTM problem.

Strategy (per spec sharding hint): 32 independent trees -> 4 trees per
NeuronCore, weights replicated, logits gathered on host.

Per-core computation is restructured around a host-computed "slot layout":
every level's nodes are permuted so that the children of each 128-parent
tile sit in a contiguous, chunk-aligned window of the next level's node
order.  The irregular-fan-in segment_sum then becomes a handful of
128x128 one-hot selector matmuls accumulating in PSUM, and all LSTM state
stays resident in SBUF in fp16 (no DRAM traffic for states at all).

Key optimizations (in order of impact):
 - **Leaf folding**: a leaf's (h, c) is a pure function of its token, so
   the whole leaf level (half of all nodes) is precomputed on the host
   into a [VOCAB, 512] fp16 table [h | c~] and row-gathered on device --
   no leaf-level compute at all.
 - The x @ W_iou^T product for non-leaf nodes is folded into the
   embedding table on the host (emb_iou, fp16, pre-scaled by SCALE), so
   their only data-dependent DMA is a row gather of iou preactivations.
 - The u-slice of W/U/b is pre-scaled by 2 on the host and c~ = c/2 is
   tracked on device, which turns tanh(u) into an affine image of
   sigmoid(2u): ONE sigmoid per node tile covers i, o and u, and
   tanh(c) = tanh(2*c~) is an activation-scale.  c~ stays exact through
   the linear child-sum recursion.
 - U_f runs as an fp8e4m3 DoubleRow matmul (256-contraction in one
   instruction at 0.5 cycles/row); weights carry x SU and the
   transposed-h operand x SH, undone by the f-sigmoid's 1/SCALE.  The
   f-gate path tolerates fp8 (~5e-4 rel err); U_iou and the x path do
   NOT (tested ~3e-2) and stay fp16.
 - Engine assignment follows per-PHASE occupancy: during the leaf /
   level-10 startup Pool is busy with SWDGE descriptor generation, so
   leaf fc-muls and fp8 conversions run on the otherwise-idle DVE;
   for levels 8-9 DVE is the bottleneck, so fc/h-muls go to GPSIMD.
 - tanh(c) is batched over quads of node tiles; selector matrices are
   fetched in 12-window chunks, small-level selectors and emb_iou rows
   in one hoisted prefetch emitted mid-pipeline (level 9) to keep the
   startup DMA window clear for the leaf-table gathers.
"""

import numpy as np
import ml_dtypes

import concourse.bacc as bacc
import concourse.bass as bass  # noqa: F401
import concourse.mybir as mybir
import concourse.tile as tile
from concourse.bass_utils import run_bass_kernel_spmd
from concourse.masks import make_identity

# ---- static problem structure (from the reference nn.Module) ----
B = 32
DEPTH = 12
LEVELS = [2 ** d for d in range(DEPTH)]
_STARTS = [0]
for _l in LEVELS[:-1]:
    _STARTS.append(_STARTS[-1] + _l)
NPG = sum(LEVELS)              # 4095 nodes per tree
N = B * NPG
VOCAB = 20000
PAD = VOCAB - 1
E = 256
H = 256
NCLS = 104
NCORES = 8
TPC = B // NCORES              # trees per core
F8 = mybir.dt.float8e4
F16 = mybir.dt.float16
F32 = mybir.dt.float32
I16 = mybir.dt.int16
GATHER_GROUP = 6               # node tiles per xsm prefetch gather
XGG = 6                        # node tiles per emb_iou gather group
USE_HT_GATHER = False           # fetch leaf h pre-transposed via dma_gather
SU = 16.0                      # fp8 U_iou / U_f weight scale
SH = 8.0                       # fp8 transposed-h operand scale
SCALE = SU * SH                # combined PSUM preact scale
SIG = mybir.ActivationFunctionType.Sigmoid
TANH = mybir.ActivationFunctionType.Tanh
COPYF = mybir.ActivationFunctionType.Copy
DR = mybir.MatmulPerfMode.DoubleRow
ADD = mybir.AluOpType.add
MULT = mybir.AluOpType.mult


# --------------------------------------------------------------------------
# host-side slot layout
# --------------------------------------------------------------------------

def _core_layout(x, par, core):
    """Per-core slot assignment. Returns per level: node-id per slot and the
    window size m (see module docstring)."""
    trees = range(core * TPC, (core + 1) * TPC)
    slot_nodes = [np.array([g * NPG for g in trees], dtype=np.int64)]
    levels = []
    for d in range(DEPTH):
        nodes = slot_nodes[d]
        Lpad = ((len(nodes) + 127) // 128) * 128
        if Lpad > len(nodes):
            nodes = np.concatenate([nodes, np.full(Lpad - len(nodes), -1,
                                                   np.int64)])
        slot_nodes[d] = nodes
        T = Lpad // 128
        lv = {"nodes": nodes, "T": T, "L": Lpad, "m": 0}
        levels.append(lv)
        if d == DEPTH - 1:
            continue
        # children of every real node in this level, grouped by parent
        ch_by_parent = {}
        for g in trees:
            lo = g * NPG + _STARTS[d + 1]
            hi = lo + LEVELS[d + 1]
            p = np.asarray(par[lo:hi])
            order = np.argsort(p, kind="stable")
            ids = np.arange(lo, hi, dtype=np.int64)[order]
            ps = p[order]
            uniq, start_idx = np.unique(ps, return_index=True)
            bounds = list(start_idx) + [len(ps)]
            for i, u in enumerate(uniq):
                ch_by_parent[int(u)] = ids[bounds[i]:bounds[i + 1]]
        cursor = 0
        child_slots = []
        m = 2
        for t in range(T):
            start = max(256 * t, cursor)
            child_slots.extend([-1] * (start - cursor))
            kids = []
            for s in range(128):
                node = nodes[t * 128 + s]
                if node >= 0:
                    kids.extend(ch_by_parent.get(int(node), ()))
            child_slots.extend(kids)
            cursor = start + len(kids)
            if cursor > 256 * t:
                m = max(m, -(-(cursor - 256 * t) // 128))
        lv["m"] = m
        slot_nodes.append(np.array(child_slots, dtype=np.int64))
    return levels


def build_layouts(x, par):
    x = np.asarray(x)
    par = np.asarray(par)
    cores = [_core_layout(x, par, c) for c in range(NCORES)]
    Ls = [max(cores[c][d]["L"] for c in range(NCORES)) for d in range(DEPTH)]
    Ts = [L // 128 for L in Ls]
    ms = [max(cores[c][d]["m"] for c in range(NCORES)) for d in range(DEPTH)]
    out = {"L": Ls, "T": Ts, "m": ms, "cores": []}
    for c in range(NCORES):
        lvs = cores[c]
        node_slots = []
        xtok = []
        for d in range(DEPTH):
            nodes = lvs[d]["nodes"]
            if len(nodes) < Ls[d]:
                nodes = np.concatenate(
                    [nodes, np.full(Ls[d] - len(nodes), -1, np.int64)])
            node_slots.append(nodes)
            tok = np.where(nodes >= 0, np.asarray(x)[np.maximum(nodes, 0)],
                           PAD)
            xtok.append(tok.astype(np.int16))
        relslot = []
        for d in range(DEPTH - 1):
            T, m = Ts[d], ms[d]
            child_nodes = node_slots[d + 1]
            rel = np.full((T, m, 128), -1.0, np.float16)
            pslot_of = {int(n): i for i, n in enumerate(node_slots[d])
                        if n >= 0}
            nch = Ls[d + 1] // 128
            for j in range(nch):
                for p in range(128):
                    node = child_nodes[j * 128 + p]
                    if node < 0:
                        continue
                    ps = pslot_of[int(par[node])]
                    t, sl = ps // 128, ps % 128
                    w = j - 2 * t
                    assert 0 <= w < m
                    rel[t, w, p] = sl
            relslot.append(rel)
        out["cores"].append({"xtok": xtok, "relslot": relslot,
                             "nodes": node_slots})
    return out


# --------------------------------------------------------------------------
# device program
# --------------------------------------------------------------------------

def _wrap16(tok):
    """int16 [L] -> the dma_gather 16-partition wrapped layout [16, L//16]."""
    return tok.reshape(-1, 16).T.copy()


def _ilhs(gt_ap, ni, off):
    """DoubleRow lhsT view [128, 2, 128] into a [128, 2, ni] fp8
    transpose-gather tile.  The gather writes, per partition p, an
    idx-major byte stream: byte q holds dim (2p + q%2) of idx q//2, so
    nodes off..off+127 occupy the 256 contiguous bytes at 2*off; the DR
    k-tile axis is the byte parity."""
    jj, oo = divmod(2 * off, ni)
    return gt_ap[:, jj, oo:oo + 256].rearrange("p (m b) -> p b m", b=2)


def build_program(Ls, Ts, ms, with_biou, with_ufb, with_linb):
    nch_of = [Ls[d] // 128 for d in range(DEPTH)]
    idx_cols = [Ls[d] // 16 for d in range(DEPTH)]
    idx_off = np.concatenate([[0], np.cumsum(idx_cols)]).astype(int)
    sel_cnt = [Ts[d] * ms[d] for d in range(DEPTH - 1)]
    sel_off = np.concatenate([[0], np.cumsum(sel_cnt)]).astype(int)
    nsel = int(sel_off[-1])

    nc = bacc.Bacc("TRN2", debug=False, num_devices=NCORES)

    emb8 = nc.dram_tensor("emb8", [VOCAB, 3 * H], F16,
                          kind="ExternalInput").ap()
    hc8 = nc.dram_tensor("hc8", [VOCAB, 2 * H], F16,
                         kind="ExternalInput").ap()
    ht8 = nc.dram_tensor("ht8", [VOCAB, H], F8,
                         kind="ExternalInput").ap()
    ufT_il = nc.dram_tensor("ufT_il", [128, 2, H], F8,
                            kind="ExternalInput").ap()
    uiou8T = nc.dram_tensor("uiou8T", [2, 128, 3 * H], F16,
                            kind="ExternalInput").ap()
    ufT = nc.dram_tensor("ufT", [2, 128, H], F8,
                         kind="ExternalInput").ap()
    linT = nc.dram_tensor("linT", [2, 128, NCLS], F16,
                          kind="ExternalInput").ap()
    biases = nc.dram_tensor("biases", [1, 3 * H + H + NCLS], F16,
                            kind="ExternalInput").ap()
    xtok = nc.dram_tensor("xtok", [128, int(idx_off[-1])], I16,
                          kind="ExternalInput").ap()
    selmat = nc.dram_tensor("selmat", [128, max(nsel, 1), 128], F16,
                            kind="ExternalInput").ap()
    logits_t = nc.dram_tensor("logits_t", [NCLS, 128], F32,
                              kind="ExternalOutput").ap()

    with tile.TileContext(nc) as tc:
        with (
            tc.tile_pool(name="const", bufs=1) as cpool,
            tc.tile_pool(name="state", bufs=1) as spool,
            tc.tile_pool(name="hcg", bufs=7) as hcpool,
            tc.tile_pool(name="hT", bufs=3) as htpool,
            tc.tile_pool(name="xg", bufs=2) as xpool,
            tc.tile_pool(name="sio", bufs=7) as siopool,
            tc.tile_pool(name="sel", bufs=4) as selpool,
            tc.tile_pool(name="work", bufs=6) as wpool,
            tc.tile_pool(name="chunk", bufs=4) as chpool,
            tc.tile_pool(name="psel", bufs=2, space="PSUM") as psel_pool,
            tc.tile_pool(name="p256", bufs=2, space="PSUM") as p256_pool,
            tc.tile_pool(name="piou", bufs=2, space="PSUM") as piou_pool,
        ):
            # ---- resident constants ----
            uiou_sb = cpool.tile([128, 2, 3 * H], F16)
            uf_sb = cpool.tile([128, 2, H], F8)
            uf_il_sb = cpool.tile([128, 2, H], F8)
            nc.sync.dma_start(uf_il_sb[:], ufT_il[:])
            lin_sb = cpool.tile([128, 2, NCLS], F16)
            bias_sb = cpool.tile([1, 3 * H + H + NCLS], F16)
            ones_sb = cpool.tile([1, 128], F16)
            idx_sb = cpool.tile([128, int(idx_off[-1])], I16)
            nc.sync.dma_start(idx_sb[:], xtok[:])
            for k in range(2):
                nc.sync.dma_start(uiou_sb[:, k, :], uiou8T[k])
                nc.sync.dma_start(uf_sb[:, k, :], ufT[k])
                nc.sync.dma_start(lin_sb[:, k, :], linT[k])
            nc.sync.dma_start(bias_sb[:], biases[:])
            nc.vector.memset(ones_sb[:], 1.0)
            ident = cpool.tile([128, 128], F16)
            make_identity(nc, ident[:])

            # ---- hoisted prefetch for the small top levels ----
            SMALL_IDX = 1536   # first 1536 slots = levels 0..7 here
            small_set = set(d for d in range(DEPTH)
                            if int(idx_off[d + 1]) * 16 <= SMALL_IDX)
            n_sm_g = (min(SMALL_IDX, int(idx_off[-1]) * 16)
                      + 767) // 768
            xsm = []
            nsel_sm = int(sel_off[max(small_set) + 1]) if small_set else 0
            s3sm = (cpool.tile([128, nsel_sm, 128], F16, name="s3sm")
                    if nsel_sm else None)

            def small_prefetch():
                """Emitted after the leaf level so these bulk transfers do
                not contend with the first leaf gathers on the DMA engines."""
                for g in range(n_sm_g):
                    xs = cpool.tile([128, GATHER_GROUP, 3 * H], F16,
                                    name=f"xsm{g}")
                    nc.gpsimd.dma_gather(
                        xs[:, :, :], emb8[:],
                        idx_sb[:, g * 48:(g + 1) * 48],
                        768, 768, 3 * H, transpose=False)
                    xsm.append(xs)
                if nsel_sm:
                    nc.sync.dma_start(s3sm[:], selmat[:, 0:nsel_sm, :])

            state = {}   # level -> (h quads, c quads), each [128, 4, H]
            SELCH = 12   # selectors per fetch chunk

            def transpose256(src_ap, tag):
                """[128,256] fp16 -> [128,256] fp16 holding the two
                transposed 128x128 halves side by side."""
                pt = p256_pool.tile([128, 256], F16, tag="p256", name="pt")
                nc.tensor.transpose(pt[:, 0:128], src_ap[:, 0:128], ident[:])
                nc.tensor.transpose(pt[:, 128:256], src_ap[:, 128:256],
                                    ident[:])
                dst = wpool.tile([128, 256], F16, tag=tag, bufs=1,
                                 name=tag)
                nc.vector.tensor_copy(dst[:], pt[:])
                return dst

            def matmul_group(out_ap, pairs):
                """Emit an accumulation group into one psum zero-region."""
                for i, (lhsT, rhs) in enumerate(pairs):
                    nc.tensor.matmul(out_ap, lhsT, rhs, start=(i == 0),
                                     stop=(i == len(pairs) - 1))

            # state slots per parity class: enough quad-tiles for the
            # largest level of that parity (the leaf level has no state
            # tiles: its (h, c~) arrives pre-computed via dma_gather)
            state_bufs = [0, 0]
            for d in range(DEPTH - 1):
                state_bufs[d % 2] = max(state_bufs[d % 2],
                                        (Ts[d] + 3) // 4)

            HCG = 6              # leaf hc row-gather group (tiles)
            HTCH = 2048          # idx per transposed-h gather chunk

            for d in range(DEPTH - 1, -1, -1):
                L, T, m = Ls[d], Ts[d], ms[d]
                if d == DEPTH - 1:
                    # ---- leaf level: (h, c~) gathered from the host-folded
                    # per-token table; h additionally gathered transposed
                    # (feature-major) for the U_f lhsT of the level above.
                    lbase = int(idx_off[d])
                    hcg, hTc = {}, {}

                    def issue_hcg(g, hcg=hcg, lbase=lbase, T=T):
                        if g < 0 or g >= -(-T // HCG) or g in hcg:
                            return
                        gt = min(HCG, T - g * HCG)
                        ht_ = hcpool.tile([128, gt, 2 * H], F16,
                                          tag="hcg", name="hcg")
                        nc.gpsimd.dma_gather(
                            ht_[:, :, :], hc8[:],
                            idx_sb[:, lbase + g * HCG * 8:
                                   lbase + g * HCG * 8 + gt * 8],
                            gt * 128, gt * 128, 2 * H, transpose=False)
                        hcg[g] = ht_

                    def issue_hT(k, hTc=hTc, lbase=lbase, T=T):
                        if k < 0 or k >= -(-(T * 128) // HTCH) or k in hTc:
                            return
                        ni = min(HTCH, T * 128 - k * HTCH)
                        tt_ = htpool.tile([128, 2, ni], F8, tag="hT",
                                          name="hT")
                        nc.gpsimd.dma_gather(
                            tt_[:, :, :], ht8[:],
                            idx_sb[:, lbase + k * (HTCH // 16):
                                   lbase + (k * HTCH + ni) // 16],
                            ni, ni, H, transpose=True)
                        hTc[k] = tt_

                    issue_hcg(0)
                    if USE_HT_GATHER:
                        issue_hT(0)
                    issue_hcg(1)
                    if not USE_HT_GATHER:
                        def issue_hT(k):  # noqa: F811
                            pass
                    state[d] = ("leaf", hcg, hTc, issue_hcg, issue_hT)
                    continue
                if d == 9:
                    small_prefetch()
                nquad = (T + 3) // 4
                h_lv = [spool.tile([128, 4, H], F16, tag=f"h{d % 2}",
                                   bufs=state_bufs[d % 2],
                                   name=f"h{d}_{q}") for q in range(nquad)]
                c_lv = [spool.tile([128, 4, H], F16, tag=f"c{d % 2}",
                                   bufs=state_bufs[d % 2],
                                   name=f"c{d}_{q}") for q in range(nquad)]
                sel_ch = {}
                nselch = (-(-(T * m) // SELCH)
                          if (d < DEPTH - 1 and d not in small_set) else 0)

                def issue_selch(ci, d=d, sel_ch=sel_ch, nselch=nselch):
                    if ci >= nselch or ci in sel_ch:
                        return
                    cnt = min(SELCH, T * m - ci * SELCH)
                    st = selpool.tile([128, cnt, 128], F16, tag="sel",
                                      name="selch")
                    base = int(sel_off[d]) + ci * SELCH
                    nc.sync.dma_start(st[:], selmat[:, base:base + cnt, :])
                    sel_ch[ci] = st
                if d < DEPTH - 1:
                    child = state[d + 1]
                    leafch = isinstance(child, tuple) and child[0] == "leaf"
                    nch = nch_of[d + 1]
                    if leafch:
                        _, hcg, hTc, issue_hcg, issue_hT = child

                        def h_half(j, k, hcg=hcg):
                            return hcg[j // HCG][:, j % HCG,
                                                 k * 128:(k + 1) * 128]

                        def h_full(j, hcg=hcg):
                            return hcg[j // HCG][:, j % HCG, 0:H]

                        def c_rows(p, n, hcg=hcg):
                            g, r = (2 * p) // HCG, (2 * p) % HCG
                            return hcg[g][:, r:r + n, H:2 * H]

                        def hT_lhs(j, hTc=hTc):
                            ck, off = divmod(j * 128, HTCH)
                            ni = min(HTCH, Ls[DEPTH - 1] - ck * HTCH)
                            return _ilhs(hTc[ck][:, :, :], ni, off)
                    else:
                        h_ch, c_ch = child

                        def h_half(j, k, h_ch=h_ch):
                            return h_ch[j // 4][:, j % 4,
                                               k * 128:(k + 1) * 128]

                        def h_full(j, h_ch=h_ch):
                            return h_ch[j // 4][:, j % 4, :]

                        def c_rows(p, n, c_ch=c_ch):
                            q, r = (2 * p) // 4, (2 * p) % 4
                            return c_ch[q][:, r:r + n, :]

                        hT_lhs = None
                    chunk_pt = {}   # pair -> (hjT2, js_p) after stage A
                    chunk_fc = {}   # chunk -> fc AP after stage B

                    def stageA(p):
                        """PE transposes of the pair's h + psum->sbuf copy.
                        For leaf children h^T arrives via the transposed
                        gather, so the stage is dependency-tracking only."""
                        js_p = [j for j in (2 * p, 2 * p + 1) if j < nch]
                        if not js_p:
                            return
                        if leafch and USE_HT_GATHER:
                            chunk_pt[p] = (None, js_p)
                            return
                        w = len(js_p) * H
                        pt2 = p256_pool.tile([128, w], F16, tag="p256",
                                             name="pt2")
                        for ji, j in enumerate(js_p):
                            for k in range(2):
                                nc.tensor.transpose(
                                    pt2[:, ji * H + k * 128:
                                        ji * H + (k + 1) * 128],
                                    h_half(j, k), ident[:])
                        hjT2 = chpool.tile([128, 2 * len(js_p), 128], F8,
                                           tag="hjT", bufs=3, name="hjT2")
                        nc.vector.tensor_scalar_mul(hjT2[:], pt2[:], SH)
                        chunk_pt[p] = (hjT2, js_p)

                    def stageB(p):
                        """U_f fp8 DoubleRow matmul + sigmoid + f*c."""
                        if p not in chunk_pt:
                            return
                        hjT2, js_p = chunk_pt.pop(p)
                        w = len(js_p) * H
                        pf2 = p256_pool.tile([128, w], F32, tag="p256",
                                             name="pf2")
                        for ji, j in enumerate(js_p):
                            if hjT2 is None:
                                lhsT, rhs = hT_lhs(j), uf_il_sb[:, :, :]
                            else:
                                lhsT = hjT2[:, 2 * ji:2 * ji + 2, :]
                                rhs = uf_sb[:, :, :]
                            nc.tensor.matmul(
                                pf2[:, ji * H:(ji + 1) * H], lhsT, rhs,
                                perf_mode=DR, start=True,
                                stop=not with_ufb)
                            if with_ufb:
                                nc.tensor.matmul(
                                    pf2[:, ji * H:(ji + 1) * H], ones_sb[:],
                                    bias_sb[:, 3 * H:4 * H],
                                    start=False, stop=True)
                        fj2 = chpool.tile([128, w], F16, tag="fj",
                                          bufs=2, name="fj2")
                        nc.scalar.activation(fj2[:], pf2[:], SIG,
                                             scale=1.0 / SCALE)
                        fc2 = chpool.tile([128, w], F16, tag="fcj",
                                          bufs=6, name="fc2")
                        fc_eng = (nc.gpsimd if d in (8, 9)
                                  else nc.vector)
                        fc_eng.tensor_mul(fc2[:], fj2[:],
                                          c_rows(p, len(js_p)))
                        for ji, j in enumerate(js_p):
                            chunk_fc[j] = fc2[:, ji * H:(ji + 1) * H]

                # xio for this level: hoisted prefetch for small levels,
                # grouped dma_gathers (issued inside the t-loop) otherwise
                xg = {}

                def issue_gather(g, d=d, xg=xg):
                    if d in small_set or g * XGG >= T:
                        return
                    gt = min(XGG, T - g * XGG)
                    gi = gt * 128
                    xt = xpool.tile([128, gt, 3 * H], F16, tag="xt",
                                    name="xt")
                    nc.gpsimd.dma_gather(
                        xt[:, :, :], emb8[:],
                        idx_sb[:, int(idx_off[d]) + g * XGG * 8:
                               int(idx_off[d]) + g * XGG * 8
                               + gt * 8],
                        gi, gi, 3 * H, transpose=False)
                    xg[g] = xt

                if d in small_set:
                    eb = int(idx_off[d]) * 16

                    def xslice(t, eb=eb):
                        g, off = divmod(eb + t * 128, 768)
                        return xsm[g][:, off // 128, :]
                else:
                    def xslice(t, xg=xg):
                        return xg[t // XGG][:, t % XGG, :]

                def js_of(t):
                    return [2 * t + w for w in range(m) if 2 * t + w < nch]

                def pairs_of(t):
                    return sorted({j // 2 for j in js_of(t)})

                def Sw_of(t, wi):
                    if d in small_set:
                        return s3sm[:, int(sel_off[d]) + t * m + wi, :]
                    si = t * m + wi
                    return sel_ch[si // SELCH][:, si % SELCH, :]

                selsb_q = {}
                doneA, doneB = set(), set()

                def stagesAB(tA, tB):
                    if 0 <= tA < T:
                        for p in pairs_of(tA):
                            if p not in doneA:
                                stageA(p)
                                doneA.add(p)
                    if 0 <= tB < T:
                        for p in pairs_of(tB):
                            if p not in doneB:
                                stageB(p)
                                doneB.add(p)

                def stageC(t):
                    psel = psel_pool.tile([128, 2 * H], F32, tag="psel",
                                          name="psel")
                    js = js_of(t)
                    if js:
                        nmm = 3 * len(js)
                        k = 0
                        for wi, j in enumerate(js):
                            Sw = Sw_of(t, wi)
                            nc.tensor.matmul(
                                psel[:, 0:128], h_half(j, 0), Sw,
                                start=(k == 0), stop=(k == nmm - 1))
                            k += 1
                            nc.tensor.matmul(
                                psel[:, 128:256], h_half(j, 1),
                                Sw, start=(k == 0), stop=(k == nmm - 1))
                            k += 1
                            nc.tensor.matmul(
                                psel[:, 2 * 128:2 * 128 + H], Sw,
                                chunk_fc[j][:],
                                start=(k == 0), stop=(k == nmm - 1))
                            k += 1
                    else:
                        raise AssertionError("empty selector window")
                    # h~^T halves + c_agg -> fp16 SBUF (psel retires fast;
                    # the later c~ add then runs in the DVE 2x mode)
                    selsb8 = wpool.tile([128, 2, 128], F16, tag="s8",
                                        bufs=3, name="s8")
                    caggsb = wpool.tile([128, 256], F16, tag="cagg",
                                        bufs=3, name="cagg")
                    nc.vector.tensor_copy(selsb8[:], psel[:, 0:256])
                    nc.vector.tensor_copy(caggsb[:], psel[:, 256:512])
                    selsb_q[t] = (selsb8, caggsb[:])

                pend = []   # (t, sio_u) awaiting the quad tanh(c) + h mul

                def flush_pend(k):
                    grp = pend[:k]
                    del pend[:k]
                    t0 = grp[0][0]
                    npr = len(grp)
                    tcn = wpool.tile([128, npr, H], F16, tag="tcn",
                                     bufs=2, name="tcn")
                    nc.scalar.activation(tcn[:],
                                         c_lv[t0 // 4][:, 0:npr, :],
                                         TANH, scale=2.0)
                    mul_eng = nc.gpsimd if d in (8, 9) else nc.vector
                    for i, (tt, sio_t) in enumerate(grp):
                        mul_eng.tensor_mul(h_lv[tt // 4][:, tt % 4, :],
                                           sio_t[:, 2 * H:3 * H],
                                           tcn[:, i, :])

                def stageDE(t):
                    sio_u = siopool.tile([128, 3 * H], F16, tag="sio",
                                         name="sio")
                    selsb8, caggs = selsb_q.pop(t)
                    pio = piou_pool.tile([128, 768], F32, tag="pio",
                                         name="pio")
                    for r0, r1 in ((0, 512), (512, 768)):
                        nc.tensor.matmul(pio[:, r0:r1], ident[:],
                                         xslice(t)[:, r0:r1],
                                         start=True, stop=False)
                        nc.tensor.matmul(pio[:, r0:r1], selsb8[:, 0, :],
                                         uiou_sb[:, 0, r0:r1],
                                         start=False, stop=False)
                        nc.tensor.matmul(pio[:, r0:r1], selsb8[:, 1, :],
                                         uiou_sb[:, 1, r0:r1],
                                         start=False, stop=not with_biou)
                        if with_biou:
                            nc.tensor.matmul(pio[:, r0:r1], ones_sb[:],
                                             bias_sb[:, r0:r1],
                                             start=False, stop=True)
                    if T <= 4:
                        # tail levels: the (i, u) pair gates the serial c~
                        # chain; sigmoid it first, o afterwards
                        nc.scalar.activation(sio_u[:, 0:512],
                                             pio[:, 0:512], SIG,
                                             scale=1.0 / SCALE)
                        nc.scalar.activation(sio_u[:, 512:768],
                                             pio[:, 512:768], SIG,
                                             scale=1.0 / SCALE)
                    else:
                        nc.scalar.activation(sio_u[:], pio[:, 0:768], SIG,
                                             scale=1.0 / SCALE)
                    # c~ = (sig(2u) - 0.5) * sig(i)  [+ c~_agg]
                    c_t = c_lv[t // 4][:, t % 4, :]
                    nc.vector.scalar_tensor_tensor(
                        c_t, sio_u[:, H:2 * H], -0.5,
                        sio_u[:, 0:H], op0=ADD, op1=MULT)
                    if d < DEPTH - 1:
                        nc.vector.tensor_add(c_t, c_t, caggs)
                    pend.append((t, sio_u))
                    if len(pend) == 6:
                        flush_pend(4)
                    if t == T - 1:
                        while pend:
                            flush_pend(min(4, len(pend)))

                issue_selch(0)
                issue_gather(0)
                if leafch:
                    issue_hcg(2)
                issue_selch(1)
                issue_gather(1)
                if leafch:
                    for g in range(3, 6):
                        issue_hcg(g)
                for k in range(6):
                    stagesAB(k, k - 1)
                for t in range(T):
                    if t % XGG == 0 and t > 0:
                        issue_gather(t // XGG + 1)
                    issue_selch(((t + 4) * m) // SELCH)
                    if leafch:
                        issue_hcg((2 * t + 27) // HCG)
                    stagesAB(t + 6, t + 5)
                    stageC(t)
                    if t >= 1:
                        stageDE(t - 1)
                stageDE(T - 1)
                state[d] = (h_lv, c_lv)

            # ---- final linear on the roots ----
            h0 = state[0][0][0][:, 0, :]
            hrT = transpose256(h0, "hrT")
            plin = p256_pool.tile([128, 128], F32, tag="p256", name="plin")
            pairs = [(lin_sb[:, 0, :], hrT[:, 0:128]),
                     (lin_sb[:, 1, :], hrT[:, 128:256])]
            if with_linb:
                pairs.append((bias_sb[:, 4 * H:4 * H + NCLS], ones_sb[:]))
            matmul_group(plin[0:NCLS, :], pairs)
            out_sb = cpool.tile([128, 128], F32)
            nc.vector.tensor_copy(out_sb[0:NCLS, :], plin[0:NCLS, :])
            nc.sync.dma_start(logits_t[:], out_sb[0:NCLS, :])

    nc.compile()
    return nc


# --------------------------------------------------------------------------
# host wrapper
# --------------------------------------------------------------------------

def prepare(inputs):
    """Returns ((Ls, Ts, ms, flags), in_maps)."""
    x = np.asarray(inputs["x"]).astype(np.int64)
    par = np.asarray(inputs["par"]).astype(np.int64)
    emb = np.asarray(inputs["emb"], dtype=np.float32).copy()
    emb[PAD] = 0.0
    W = np.asarray(inputs["W_iou"], np.float32).copy()
    U = np.asarray(inputs["U_iou"], np.float32).copy()
    Uf = np.asarray(inputs["U_f_w"], np.float32)
    lin = np.asarray(inputs["lin_w"], np.float32)
    b_iou = np.asarray(inputs["b_iou"], np.float32).reshape(-1).copy()
    ufb = np.asarray(inputs["U_f_b"], np.float32).reshape(-1)
    linb = np.asarray(inputs["lin_b"], np.float32).reshape(-1)

    # ---- leaf folding: a leaf's (h, c) depends only on its token, so the
    # whole leaf-level LSTM cell is precomputed per vocab entry on the host.
    # Table rows are [h_leaf | c_leaf/2] (c~ = c/2 is what the device
    # tracks).
    iou_leaf = emb @ W.T + b_iou[None, :]
    il, ol, ul = (iou_leaf[:, 0:H], iou_leaf[:, H:2 * H],
                  iou_leaf[:, 2 * H:3 * H])
    sig = lambda v: 1.0 / (1.0 + np.exp(-v))  # noqa: E731
    c_leaf = sig(il) * np.tanh(ul)
    h_leaf = sig(ol) * np.tanh(c_leaf)
    hc8 = np.concatenate([h_leaf, 0.5 * c_leaf], axis=1).astype(np.float16)
    # leaf h again as a scaled fp8 table for the transposed gather feeding
    # the U_f DoubleRow lhsT (the 16-bit-granularity transpose interleaves
    # fp8 pairs: partition p holds dims (2p, 2p+1))
    ht8 = (h_leaf * SH).astype(ml_dtypes.float8_e4m3)

    # fold the x @ W_iou^T product into the embedding table; pre-scale the
    # u-slice by 2 (the device tracks c~ = c/2 and computes
    # tanh(u) via 2*sigmoid(2u) - 1)
    W[2 * H:3 * H] *= 2.0
    U[2 * H:3 * H] *= 2.0
    b_iou[2 * H:3 * H] *= 2.0
    # U_iou / U_f run as fp8e4m3 DoubleRow matmuls.  fp8's dynamic range
    # bottoms out near these weights' natural ~0.05 magnitude, so the
    # weights carry a x SU scale and the transposed-h operands a x SH
    # scale; the iou/f sigmoids read PSUM with scale 1/(SU*SH).  The
    # x-side preacts (emb8) and biases are pre-scaled to match.
    # column order [i | u | o]: the u-slice rides next to i so the tail
    # levels can sigmoid the chain-critical (i, u) pair first
    iuo = np.concatenate([np.arange(0, H), np.arange(2 * H, 3 * H),
                          np.arange(H, 2 * H)])
    emb8 = (emb @ W.T * SCALE)[:, iuo].astype(np.float16)

    lay = build_layouts(x, par)
    Ls, Ts, ms = lay["L"], lay["T"], lay["m"]

    uiou8T = np.ascontiguousarray(
        (U.T * SCALE)[:, iuo].reshape(2, 128, 3 * H)).astype(np.float16)
    ufT = np.ascontiguousarray(Uf.T.reshape(2, 128, H) * SU).astype(
        ml_dtypes.float8_e4m3)
    # interleaved-row variant matching the fp8 transposed-gather layout:
    # (p, j) holds U_f^T row 2p+j
    ufT_il = np.ascontiguousarray(Uf.T.reshape(128, 2, H) * SU).astype(
        ml_dtypes.float8_e4m3)
    linT = np.ascontiguousarray(lin.T.reshape(2, 128, NCLS)).astype(
        np.float16)
    biases = np.concatenate([(b_iou * SCALE)[iuo], ufb * SCALE,
                             linb]).astype(np.float16)[None, :]

    flags = dict(with_biou=bool(np.any(b_iou)), with_ufb=bool(np.any(ufb)),
                 with_linb=bool(np.any(linb)))

    in_maps = []
    for c in range(NCORES):
        cl = lay["cores"][c]
        xtokc = np.concatenate([_wrap16(cl["xtok"][d]) for d in range(DEPTH)],
                               axis=1)
        xtokc = np.tile(xtokc, (8, 1))  # replicate across the 8 Q7 cores
        nsel = sum(Ts[d] * ms[d] for d in range(DEPTH - 1))
        rel = np.concatenate(
            [cl["relslot"][d].reshape(-1, 128) for d in range(DEPTH - 1)],
            axis=0)  # [nsel, 128] float16 rel slot per (sel, child-part)
        # one-hot selector matrices, laid out [child_part, sel, parent_slot]
        sel1h = (rel[:, :, None] ==
                 np.arange(128, dtype=np.float32)[None, None, :])
        selm = np.ascontiguousarray(
            sel1h.transpose(1, 0, 2)).astype(np.float16)
        if nsel == 0:
            selm = np.zeros((128, 1, 128), np.float16)
        in_maps.append({
            "emb8": emb8,
            "hc8": hc8,
            "ht8": ht8,
            "uiou8T": uiou8T,
            "ufT": ufT,
            "ufT_il": ufT_il,
            "linT": linT,
            "biases": biases,
            "xtok": np.ascontiguousarray(xtokc).astype(np.int16),
            "selmat": selm,
        })
    return (Ls, Ts, ms, flags), in_maps


_PROGRAM_CACHE = {}


def get_program(Ls, Ts, ms, flags):
    key = (tuple(Ls), tuple(ms), tuple(sorted(flags.items())))
    if key not in _PROGRAM_CACHE:
        _PROGRAM_CACHE[key] = build_program(Ls, Ts, ms, **flags)
    return _PROGRAM_CACHE[key]


def _ensure_device_backend():
    """run_bass_kernel_spmd executes via jax.devices(); make sure those are
    the NeuronCores, not a host-emulation platform."""
    import jax
    try:
        if jax.devices()[0].platform in ("neuron", "axon"):
            return
    except Exception:
        pass
    try:
        jax.config.update("jax_platforms", "neuron")
        jax.clear_backends()
        assert jax.devices()[0].platform in ("neuron", "axon")
    except Exception:
        pass


def kernel(**inputs):
    _ensure_device_backend()
    (Ls, Ts, ms, flags), in_maps = prepare(inputs)
    nc = get_program(Ls, Ts, ms, flags)
    res = run_bass_kernel_spmd(nc, in_maps, core_ids=list(range(NCORES)))
    logits = np.zeros((B, NCLS), np.float32)
    for c in range(NCORES):
        lt = res.results[c]["logits_t"]  # [104, 128]
        logits[c * TPC:(c + 1) * TPC] = np.asarray(lt)[:, 0:TPC].T
    return logits



# revision 76
# speedup vs baseline: 1.0244x; 1.0063x over previous
"""Trainium2 Bass kernel for the DGL ChildSum-TreeLSTM problem.

Strategy (per spec sharding hint): 32 independent trees -> 4 trees per
NeuronCore, weights replicated, logits gathered on host.

Per-core computation is restructured around a host-computed "slot layout":
every level's nodes are permuted so that the children of each 128-parent
tile sit in a contiguous, chunk-aligned window of the next level's node
order.  The irregular-fan-in segment_sum then becomes a handful of
128x128 one-hot selector matmuls accumulating in PSUM, and all LSTM state
stays resident in SBUF in fp16 (no DRAM traffic for states at all).

Key optimizations (in order of impact):
 - **Leaf folding**: a leaf's (h, c) is a pure function of its token, so
   the whole leaf level (half of all nodes) is precomputed on the host
   into a [VOCAB, 512] fp16 table [h | c~] and row-gathered on device --
   no leaf-level compute at all.
 - The x @ W_iou^T product for non-leaf nodes is folded into the
   embedding table on the host (emb_iou, fp16, pre-scaled by SCALE), so
   their only data-dependent DMA is a row gather of iou preactivations.
 - The u-slice of W/U/b is pre-scaled by 2 on the host and c~ = c/2 is
   tracked on device, which turns tanh(u) into an affine image of
   sigmoid(2u): ONE sigmoid per node tile covers i, o and u, and
   tanh(c) = tanh(2*c~) is an activation-scale.  c~ stays exact through
   the linear child-sum recursion.
 - U_f runs as an fp8e4m3 DoubleRow matmul (256-contraction in one
   instruction at 0.5 cycles/row); weights carry x SU and the
   transposed-h operand x SH, undone by the f-sigmoid's 1/SCALE.  The
   f-gate path tolerates fp8 (~5e-4 rel err); U_iou and the x path do
   NOT (tested ~3e-2) and stay fp16.
 - Engine assignment follows per-PHASE occupancy: during the leaf /
   level-10 startup Pool is busy with SWDGE descriptor generation, so
   leaf fc-muls and fp8 conversions run on the otherwise-idle DVE;
   for levels 8-9 DVE is the bottleneck, so fc/h-muls go to GPSIMD.
 - tanh(c) is batched over quads of node tiles; selector matrices are
   fetched in 12-window chunks, small-level selectors and emb_iou rows
   in one hoisted prefetch emitted mid-pipeline (level 9) to keep the
   startup DMA window clear for the leaf-table gathers.
"""

import numpy as np
import ml_dtypes

import concourse.bacc as bacc
import concourse.bass as bass  # noqa: F401
import concourse.mybir as mybir
import concourse.tile as tile
from concourse.bass_utils import run_bass_kernel_spmd
from concourse.masks import make_identity

# ---- static problem structure (from the reference nn.Module) ----
B = 32
DEPTH = 12
LEVELS = [2 ** d for d in range(DEPTH)]
_STARTS = [0]
for _l in LEVELS[:-1]:
    _STARTS.append(_STARTS[-1] + _l)
NPG = sum(LEVELS)              # 4095 nodes per tree
N = B * NPG
VOCAB = 20000
PAD = VOCAB - 1
E = 256
H = 256
NCLS = 104
NCORES = 8
TPC = B // NCORES              # trees per core
F8 = mybir.dt.float8e4
F16 = mybir.dt.float16
F32 = mybir.dt.float32
I16 = mybir.dt.int16
GATHER_GROUP = 6               # node tiles per xsm prefetch gather
XGG = 6                        # node tiles per emb_iou gather group
USE_HT_GATHER = False           # fetch leaf h pre-transposed via dma_gather
SU = 16.0                      # fp8 U_iou / U_f weight scale
SH = 8.0                       # fp8 transposed-h operand scale
SCALE = SU * SH                # combined PSUM preact scale
SIG = mybir.ActivationFunctionType.Sigmoid
TANH = mybir.ActivationFunctionType.Tanh
COPYF = mybir.ActivationFunctionType.Copy
DR = mybir.MatmulPerfMode.DoubleRow
ADD = mybir.AluOpType.add
MULT = mybir.AluOpType.mult


# --------------------------------------------------------------------------
# host-side slot layout
# --------------------------------------------------------------------------

def _core_layout(x, par, core):
    """Per-core slot assignment. Returns per level: node-id per slot and the
    window size m (see module docstring)."""
    trees = range(core * TPC, (core + 1) * TPC)
    slot_nodes = [np.array([g * NPG for g in trees], dtype=np.int64)]
    levels = []
    for d in range(DEPTH):
        nodes = slot_nodes[d]
        Lpad = ((len(nodes) + 127) // 128) * 128
        if Lpad > len(nodes):
            nodes = np.concatenate([nodes, np.full(Lpad - len(nodes), -1,
                                                   np.int64)])
        slot_nodes[d] = nodes
        T = Lpad // 128
        lv = {"nodes": nodes, "T": T, "L": Lpad, "m": 0}
        levels.append(lv)
        if d == DEPTH - 1:
            continue
        # children of every real node in this level, grouped by parent
        ch_by_parent = {}
        for g in trees:
            lo = g * NPG + _STARTS[d + 1]
            hi = lo + LEVELS[d + 1]
            p = np.asarray(par[lo:hi])
            order = np.argsort(p, kind="stable")
            ids = np.arange(lo, hi, dtype=np.int64)[order]
            ps = p[order]
            uniq, start_idx = np.unique(ps, return_index=True)
            bounds = list(start_idx) + [len(ps)]
            for i, u in enumerate(uniq):
                ch_by_parent[int(u)] = ids[bounds[i]:bounds[i + 1]]
        cursor = 0
        child_slots = []
        m = 2
        for t in range(T):
            start = max(256 * t, cursor)
            child_slots.extend([-1] * (start - cursor))
            kids = []
            for s in range(128):
                node = nodes[t * 128 + s]
                if node >= 0:
                    kids.extend(ch_by_parent.get(int(node), ()))
            child_slots.extend(kids)
            cursor = start + len(kids)
            if cursor > 256 * t:
                m = max(m, -(-(cursor - 256 * t) // 128))
        lv["m"] = m
        slot_nodes.append(np.array(child_slots, dtype=np.int64))
    return levels


def build_layouts(x, par):
    x = np.asarray(x)
    par = np.asarray(par)
    cores = [_core_layout(x, par, c) for c in range(NCORES)]
    Ls = [max(cores[c][d]["L"] for c in range(NCORES)) for d in range(DEPTH)]
    Ts = [L // 128 for L in Ls]
    ms = [max(cores[c][d]["m"] for c in range(NCORES)) for d in range(DEPTH)]
    out = {"L": Ls, "T": Ts, "m": ms, "cores": []}
    for c in range(NCORES):
        lvs = cores[c]
        node_slots = []
        xtok = []
        for d in range(DEPTH):
            nodes = lvs[d]["nodes"]
            if len(nodes) < Ls[d]:
                nodes = np.concatenate(
                    [nodes, np.full(Ls[d] - len(nodes), -1, np.int64)])
            node_slots.append(nodes)
            tok = np.where(nodes >= 0, np.asarray(x)[np.maximum(nodes, 0)],
                           PAD)
            xtok.append(tok.astype(np.int16))
        relslot = []
        for d in range(DEPTH - 1):
            T, m = Ts[d], ms[d]
            child_nodes = node_slots[d + 1]
            rel = np.full((T, m, 128), -1.0, np.float16)
            pslot_of = {int(n): i for i, n in enumerate(node_slots[d])
                        if n >= 0}
            nch = Ls[d + 1] // 128
            for j in range(nch):
                for p in range(128):
                    node = child_nodes[j * 128 + p]
                    if node < 0:
                        continue
                    ps = pslot_of[int(par[node])]
                    t, sl = ps // 128, ps % 128
                    w = j - 2 * t
                    assert 0 <= w < m
                    rel[t, w, p] = sl
            relslot.append(rel)
        out["cores"].append({"xtok": xtok, "relslot": relslot,
                             "nodes": node_slots})
    return out


# --------------------------------------------------------------------------
# device program
# --------------------------------------------------------------------------

def _wrap16(tok):
    """int16 [L] -> the dma_gather 16-partition wrapped layout [16, L//16]."""
    return tok.reshape(-1, 16).T.copy()


def _ilhs(gt_ap, ni, off):
    """DoubleRow lhsT view [128, 2, 128] into a [128, 2, ni] fp8
    transpose-gather tile.  The gather writes, per partition p, an
    idx-major byte stream: byte q holds dim (2p + q%2) of idx q//2, so
    nodes off..off+127 occupy the 256 contiguous bytes at 2*off; the DR
    k-tile axis is the byte parity."""
    jj, oo = divmod(2 * off, ni)
    return gt_ap[:, jj, oo:oo + 256].rearrange("p (m b) -> p b m", b=2)


def build_program(Ls, Ts, ms, with_biou, with_ufb, with_linb):
    nch_of = [Ls[d] // 128 for d in range(DEPTH)]
    idx_cols = [Ls[d] // 16 for d in range(DEPTH)]
    idx_off = np.concatenate([[0], np.cumsum(idx_cols)]).astype(int)
    sel_cnt = [Ts[d] * ms[d] for d in range(DEPTH - 1)]
    sel_off = np.concatenate([[0], np.cumsum(sel_cnt)]).astype(int)
    nsel = int(sel_off[-1])

    nc = bacc.Bacc("TRN2", debug=False, num_devices=NCORES)

    emb8 = nc.dram_tensor("emb8", [VOCAB, 3 * H], F16,
                          kind="ExternalInput").ap()
    hc8 = nc.dram_tensor("hc8", [VOCAB, 2 * H], F16,
                         kind="ExternalInput").ap()
    ht8 = nc.dram_tensor("ht8", [VOCAB, H], F8,
                         kind="ExternalInput").ap()
    ufT_il = nc.dram_tensor("ufT_il", [128, 2, H], F8,
                            kind="ExternalInput").ap()
    uiou8T = nc.dram_tensor("uiou8T", [2, 128, 3 * H], F16,
                            kind="ExternalInput").ap()
    ufT = nc.dram_tensor("ufT", [2, 128, H], F8,
                         kind="ExternalInput").ap()
    linT = nc.dram_tensor("linT", [2, 128, NCLS], F16,
                          kind="ExternalInput").ap()
    biases = nc.dram_tensor("biases", [1, 3 * H + H + NCLS], F16,
                            kind="ExternalInput").ap()
    xtok = nc.dram_tensor("xtok", [128, int(idx_off[-1])], I16,
                          kind="ExternalInput").ap()
    selmat = nc.dram_tensor("selmat", [128, max(nsel, 1), 128], F16,
                            kind="ExternalInput").ap()
    logits_t = nc.dram_tensor("logits_t", [NCLS, 128], F32,
                              kind="ExternalOutput").ap()

    with tile.TileContext(nc) as tc:
        with (
            tc.tile_pool(name="const", bufs=1) as cpool,
            tc.tile_pool(name="state", bufs=1) as spool,
            tc.tile_pool(name="hcg", bufs=7) as hcpool,
            tc.tile_pool(name="hT", bufs=3) as htpool,
            tc.tile_pool(name="xg", bufs=2) as xpool,
            tc.tile_pool(name="sio", bufs=7) as siopool,
            tc.tile_pool(name="sel", bufs=4) as selpool,
            tc.tile_pool(name="work", bufs=6) as wpool,
            tc.tile_pool(name="chunk", bufs=4) as chpool,
            tc.tile_pool(name="psel", bufs=2, space="PSUM") as psel_pool,
            tc.tile_pool(name="p256", bufs=2, space="PSUM") as p256_pool,
            tc.tile_pool(name="piou", bufs=2, space="PSUM") as piou_pool,
        ):
            # ---- resident constants ----
            uiou_sb = cpool.tile([128, 2, 3 * H], F16)
            uf_sb = cpool.tile([128, 2, H], F8)
            uf_il_sb = cpool.tile([128, 2, H], F8)
            nc.sync.dma_start(uf_il_sb[:], ufT_il[:])
            lin_sb = cpool.tile([128, 2, NCLS], F16)
            bias_sb = cpool.tile([1, 3 * H + H + NCLS], F16)
            ones_sb = cpool.tile([1, 128], F16)
            idx_sb = cpool.tile([128, int(idx_off[-1])], I16)
            nc.sync.dma_start(idx_sb[:], xtok[:])
            for k in range(2):
                nc.sync.dma_start(uiou_sb[:, k, :], uiou8T[k])
                nc.sync.dma_start(uf_sb[:, k, :], ufT[k])
                nc.sync.dma_start(lin_sb[:, k, :], linT[k])
            nc.sync.dma_start(bias_sb[:], biases[:])
            nc.vector.memset(ones_sb[:], 1.0)
            ident = cpool.tile([128, 128], F16)
            make_identity(nc, ident[:])

            # ---- hoisted prefetch for the small top levels ----
            SMALL_IDX = 1536   # first 1536 slots = levels 0..7 here
            small_set = set(d for d in range(DEPTH)
                            if int(idx_off[d + 1]) * 16 <= SMALL_IDX)
            n_sm_g = (min(SMALL_IDX, int(idx_off[-1]) * 16)
                      + 767) // 768
            xsm = []
            nsel_sm = int(sel_off[max(small_set) + 1]) if small_set else 0
            s3sm = (cpool.tile([128, nsel_sm, 128], F16, name="s3sm")
                    if nsel_sm else None)

            def small_prefetch():
                """Emitted after the leaf level so these bulk transfers do
                not contend with the first leaf gathers on the DMA engines."""
                for g in range(n_sm_g):
                    xs = cpool.tile([128, GATHER_GROUP, 3 * H], F16,
                                    name=f"xsm{g}")
                    nc.gpsimd.dma_gather(
                        xs[:, :, :], emb8[:],
                        idx_sb[:, g * 48:(g + 1) * 48],
                        768, 768, 3 * H, transpose=False)
                    xsm.append(xs)
                if nsel_sm:
                    nc.sync.dma_start(s3sm[:], selmat[:, 0:nsel_sm, :])

            state = {}   # level -> (h quads, c quads), each [128, 4, H]
            SELCH = 8    # selectors per fetch chunk

            def transpose256(src_ap, tag):
                """[128,256] fp16 -> [128,256] fp16 holding the two
                transposed 128x128 halves side by side."""
                pt = p256_pool.tile([128, 256], F16, tag="p256", name="pt")
                nc.tensor.transpose(pt[:, 0:128], src_ap[:, 0:128], ident[:])
                nc.tensor.transpose(pt[:, 128:256], src_ap[:, 128:256],
                                    ident[:])
                dst = wpool.tile([128, 256], F16, tag=tag, bufs=1,
                                 name=tag)
                nc.vector.tensor_copy(dst[:], pt[:])
                return dst

            def matmul_group(out_ap, pairs):
                """Emit an accumulation group into one psum zero-region."""
                for i, (lhsT, rhs) in enumerate(pairs):
                    nc.tensor.matmul(out_ap, lhsT, rhs, start=(i == 0),
                                     stop=(i == len(pairs) - 1))

            # state slots per parity class: enough quad-tiles for the
            # largest level of that parity (the leaf level has no state
            # tiles: its (h, c~) arrives pre-computed via dma_gather)
            state_bufs = [0, 0]
            for d in range(DEPTH - 1):
                state_bufs[d % 2] = max(state_bufs[d % 2],
                                        (Ts[d] + 3) // 4)

            HCG = 6              # leaf hc row-gather group (tiles)
            HTCH = 2048          # idx per transposed-h gather chunk

            for d in range(DEPTH - 1, -1, -1):
                L, T, m = Ls[d], Ts[d], ms[d]
                if d == DEPTH - 1:
                    # ---- leaf level: (h, c~) gathered from the host-folded
                    # per-token table; h additionally gathered transposed
                    # (feature-major) for the U_f lhsT of the level above.
                    lbase = int(idx_off[d])
                    hcg, hTc = {}, {}

                    def issue_hcg(g, hcg=hcg, lbase=lbase, T=T):
                        if g < 0 or g >= -(-T // HCG) or g in hcg:
                            return
                        gt = min(HCG, T - g * HCG)
                        ht_ = hcpool.tile([128, gt, 2 * H], F16,
                                          tag="hcg", name="hcg")
                        nc.gpsimd.dma_gather(
                            ht_[:, :, :], hc8[:],
                            idx_sb[:, lbase + g * HCG * 8:
                                   lbase + g * HCG * 8 + gt * 8],
                            gt * 128, gt * 128, 2 * H, transpose=False)
                        hcg[g] = ht_

                    def issue_hT(k, hTc=hTc, lbase=lbase, T=T):
                        if k < 0 or k >= -(-(T * 128) // HTCH) or k in hTc:
                            return
                        ni = min(HTCH, T * 128 - k * HTCH)
                        tt_ = htpool.tile([128, 2, ni], F8, tag="hT",
                                          name="hT")
                        nc.gpsimd.dma_gather(
                            tt_[:, :, :], ht8[:],
                            idx_sb[:, lbase + k * (HTCH // 16):
                                   lbase + (k * HTCH + ni) // 16],
                            ni, ni, H, transpose=True)
                        hTc[k] = tt_

                    issue_hcg(0)
                    if USE_HT_GATHER:
                        issue_hT(0)
                    issue_hcg(1)
                    if not USE_HT_GATHER:
                        def issue_hT(k):  # noqa: F811
                            pass
                    state[d] = ("leaf", hcg, hTc, issue_hcg, issue_hT)
                    continue
                if d == 9:
                    small_prefetch()
                nquad = (T + 3) // 4
                h_lv = [spool.tile([128, 4, H], F16, tag=f"h{d % 2}",
                                   bufs=state_bufs[d % 2],
                                   name=f"h{d}_{q}") for q in range(nquad)]
                c_lv = [spool.tile([128, 4, H], F16, tag=f"c{d % 2}",
                                   bufs=state_bufs[d % 2],
                                   name=f"c{d}_{q}") for q in range(nquad)]
                sel_ch = {}
                nselch = (-(-(T * m) // SELCH)
                          if (d < DEPTH - 1 and d not in small_set) else 0)

                def issue_selch(ci, d=d, sel_ch=sel_ch, nselch=nselch):
                    if ci >= nselch or ci in sel_ch:
                        return
                    cnt = min(SELCH, T * m - ci * SELCH)
                    st = selpool.tile([128, cnt, 128], F16, tag="sel",
                                      name="selch")
                    base = int(sel_off[d]) + ci * SELCH
                    nc.sync.dma_start(st[:], selmat[:, base:base + cnt, :])
                    sel_ch[ci] = st
                if d < DEPTH - 1:
                    child = state[d + 1]
                    leafch = isinstance(child, tuple) and child[0] == "leaf"
                    nch = nch_of[d + 1]
                    if leafch:
                        _, hcg, hTc, issue_hcg, issue_hT = child

                        def h_half(j, k, hcg=hcg):
                            return hcg[j // HCG][:, j % HCG,
                                                 k * 128:(k + 1) * 128]

                        def h_full(j, hcg=hcg):
                            return hcg[j // HCG][:, j % HCG, 0:H]

                        def c_rows(p, n, hcg=hcg):
                            g, r = (2 * p) // HCG, (2 * p) % HCG
                            return hcg[g][:, r:r + n, H:2 * H]

                        def hT_lhs(j, hTc=hTc):
                            ck, off = divmod(j * 128, HTCH)
                            ni = min(HTCH, Ls[DEPTH - 1] - ck * HTCH)
                            return _ilhs(hTc[ck][:, :, :], ni, off)
                    else:
                        h_ch, c_ch = child

                        def h_half(j, k, h_ch=h_ch):
                            return h_ch[j // 4][:, j % 4,
                                               k * 128:(k + 1) * 128]

                        def h_full(j, h_ch=h_ch):
                            return h_ch[j // 4][:, j % 4, :]

                        def c_rows(p, n, c_ch=c_ch):
                            q, r = (2 * p) // 4, (2 * p) % 4
                            return c_ch[q][:, r:r + n, :]

                        hT_lhs = None
                    chunk_pt = {}   # pair -> (hjT2, js_p) after stage A
                    chunk_fc = {}   # chunk -> fc AP after stage B

                    def stageA(p):
                        """PE transposes of the pair's h + psum->sbuf copy.
                        For leaf children h^T arrives via the transposed
                        gather, so the stage is dependency-tracking only."""
                        js_p = [j for j in (2 * p, 2 * p + 1) if j < nch]
                        if not js_p:
                            return
                        if leafch and USE_HT_GATHER:
                            chunk_pt[p] = (None, js_p)
                            return
                        w = len(js_p) * H
                        pt2 = p256_pool.tile([128, w], F16, tag="p256",
                                             name="pt2")
                        for ji, j in enumerate(js_p):
                            for k in range(2):
                                nc.tensor.transpose(
                                    pt2[:, ji * H + k * 128:
                                        ji * H + (k + 1) * 128],
                                    h_half(j, k), ident[:])
                        hjT2 = chpool.tile([128, 2 * len(js_p), 128], F8,
                                           tag="hjT", bufs=3, name="hjT2")
                        nc.vector.tensor_scalar_mul(hjT2[:], pt2[:], SH)
                        chunk_pt[p] = (hjT2, js_p)

                    def stageB(p):
                        """U_f fp8 DoubleRow matmul + sigmoid + f*c."""
                        if p not in chunk_pt:
                            return
                        hjT2, js_p = chunk_pt.pop(p)
                        w = len(js_p) * H
                        pf2 = p256_pool.tile([128, w], F32, tag="p256",
                                             name="pf2")
                        for ji, j in enumerate(js_p):
                            if hjT2 is None:
                                lhsT, rhs = hT_lhs(j), uf_il_sb[:, :, :]
                            else:
                                lhsT = hjT2[:, 2 * ji:2 * ji + 2, :]
                                rhs = uf_sb[:, :, :]
                            nc.tensor.matmul(
                                pf2[:, ji * H:(ji + 1) * H], lhsT, rhs,
                                perf_mode=DR, start=True,
                                stop=not with_ufb)
                            if with_ufb:
                                nc.tensor.matmul(
                                    pf2[:, ji * H:(ji + 1) * H], ones_sb[:],
                                    bias_sb[:, 3 * H:4 * H],
                                    start=False, stop=True)
                        fj2 = chpool.tile([128, w], F16, tag="fj",
                                          bufs=2, name="fj2")
                        nc.scalar.activation(fj2[:], pf2[:], SIG,
                                             scale=1.0 / SCALE)
                        fc2 = chpool.tile([128, w], F16, tag="fcj",
                                          bufs=6, name="fc2")
                        fc_eng = (nc.gpsimd if d in (8, 9)
                                  else nc.vector)
                        fc_eng.tensor_mul(fc2[:], fj2[:],
                                          c_rows(p, len(js_p)))
                        for ji, j in enumerate(js_p):
                            chunk_fc[j] = fc2[:, ji * H:(ji + 1) * H]

                # xio for this level: hoisted prefetch for small levels,
                # grouped dma_gathers (issued inside the t-loop) otherwise
                xg = {}

                def issue_gather(g, d=d, xg=xg):
                    if d in small_set or g * XGG >= T:
                        return
                    gt = min(XGG, T - g * XGG)
                    gi = gt * 128
                    xt = xpool.tile([128, gt, 3 * H], F16, tag="xt",
                                    name="xt")
                    nc.gpsimd.dma_gather(
                        xt[:, :, :], emb8[:],
                        idx_sb[:, int(idx_off[d]) + g * XGG * 8:
                               int(idx_off[d]) + g * XGG * 8
                               + gt * 8],
                        gi, gi, 3 * H, transpose=False)
                    xg[g] = xt

                if d in small_set:
                    eb = int(idx_off[d]) * 16

                    def xslice(t, eb=eb):
                        g, off = divmod(eb + t * 128, 768)
                        return xsm[g][:, off // 128, :]
                else:
                    def xslice(t, xg=xg):
                        return xg[t // XGG][:, t % XGG, :]

                def js_of(t):
                    return [2 * t + w for w in range(m) if 2 * t + w < nch]

                def pairs_of(t):
                    return sorted({j // 2 for j in js_of(t)})

                def Sw_of(t, wi):
                    if d in small_set:
                        return s3sm[:, int(sel_off[d]) + t * m + wi, :]
                    si = t * m + wi
                    return sel_ch[si // SELCH][:, si % SELCH, :]

                selsb_q = {}
                doneA, doneB = set(), set()

                def stagesAB(tA, tB):
                    if 0 <= tA < T:
                        for p in pairs_of(tA):
                            if p not in doneA:
                                stageA(p)
                                doneA.add(p)
                    if 0 <= tB < T:
                        for p in pairs_of(tB):
                            if p not in doneB:
                                stageB(p)
                                doneB.add(p)

                def stageC(t):
                    psel = psel_pool.tile([128, 2 * H], F32, tag="psel",
                                          name="psel")
                    js = js_of(t)
                    if js:
                        nmm = 3 * len(js)
                        k = 0
                        for wi, j in enumerate(js):
                            Sw = Sw_of(t, wi)
                            nc.tensor.matmul(
                                psel[:, 0:128], h_half(j, 0), Sw,
                                start=(k == 0), stop=(k == nmm - 1))
                            k += 1
                            nc.tensor.matmul(
                                psel[:, 128:256], h_half(j, 1),
                                Sw, start=(k == 0), stop=(k == nmm - 1))
                            k += 1
                            nc.tensor.matmul(
                                psel[:, 2 * 128:2 * 128 + H], Sw,
                                chunk_fc[j][:],
                                start=(k == 0), stop=(k == nmm - 1))
                            k += 1
                    else:
                        raise AssertionError("empty selector window")
                    # h~^T halves + c_agg -> fp16 SBUF (psel retires fast;
                    # the later c~ add then runs in the DVE 2x mode)
                    selsb8 = wpool.tile([128, 2, 128], F16, tag="s8",
                                        bufs=3, name="s8")
                    caggsb = wpool.tile([128, 256], F16, tag="cagg",
                                        bufs=3, name="cagg")
                    nc.vector.tensor_copy(selsb8[:], psel[:, 0:256])
                    nc.vector.tensor_copy(caggsb[:], psel[:, 256:512])
                    selsb_q[t] = (selsb8, caggsb[:])

                pend = []   # (t, sio_u) awaiting the quad tanh(c) + h mul

                def flush_pend(k):
                    grp = pend[:k]
                    del pend[:k]
                    t0 = grp[0][0]
                    npr = len(grp)
                    tcn = wpool.tile([128, npr, H], F16, tag="tcn",
                                     bufs=2, name="tcn")
                    nc.scalar.activation(tcn[:],
                                         c_lv[t0 // 4][:, 0:npr, :],
                                         TANH, scale=2.0)
                    mul_eng = nc.gpsimd if d in (8, 9) else nc.vector
                    for i, (tt, sio_t) in enumerate(grp):
                        mul_eng.tensor_mul(h_lv[tt // 4][:, tt % 4, :],
                                           sio_t[:, 2 * H:3 * H],
                                           tcn[:, i, :])

                def stageDE(t):
                    sio_u = siopool.tile([128, 3 * H], F16, tag="sio",
                                         name="sio")
                    selsb8, caggs = selsb_q.pop(t)
                    pio = piou_pool.tile([128, 768], F32, tag="pio",
                                         name="pio")
                    for r0, r1 in ((0, 512), (512, 768)):
                        nc.tensor.matmul(pio[:, r0:r1], ident[:],
                                         xslice(t)[:, r0:r1],
                                         start=True, stop=False)
                        nc.tensor.matmul(pio[:, r0:r1], selsb8[:, 0, :],
                                         uiou_sb[:, 0, r0:r1],
                                         start=False, stop=False)
                        nc.tensor.matmul(pio[:, r0:r1], selsb8[:, 1, :],
                                         uiou_sb[:, 1, r0:r1],
                                         start=False, stop=not with_biou)
                        if with_biou:
                            nc.tensor.matmul(pio[:, r0:r1], ones_sb[:],
                                             bias_sb[:, r0:r1],
                                             start=False, stop=True)
                    if T <= 4:
                        # tail levels: the (i, u) pair gates the serial c~
                        # chain; sigmoid it first, o afterwards
                        nc.scalar.activation(sio_u[:, 0:512],
                                             pio[:, 0:512], SIG,
                                             scale=1.0 / SCALE)
                        nc.scalar.activation(sio_u[:, 512:768],
                                             pio[:, 512:768], SIG,
                                             scale=1.0 / SCALE)
                    else:
                        nc.scalar.activation(sio_u[:], pio[:, 0:768], SIG,
                                             scale=1.0 / SCALE)
                    # c~ = (sig(2u) - 0.5) * sig(i)  [+ c~_agg]
                    c_t = c_lv[t // 4][:, t % 4, :]
                    nc.vector.scalar_tensor_tensor(
                        c_t, sio_u[:, H:2 * H], -0.5,
                        sio_u[:, 0:H], op0=ADD, op1=MULT)
                    if d < DEPTH - 1:
                        nc.vector.tensor_add(c_t, c_t, caggs)
                    pend.append((t, sio_u))
                    if len(pend) == 6:
                        flush_pend(4)
                    if t == T - 1:
                        while pend:
                            flush_pend(min(4, len(pend)))

                issue_selch(0)
                issue_gather(0)
                if leafch:
                    issue_hcg(2)
                issue_selch(1)
                issue_gather(1)
                if leafch:
                    for g in range(3, 6):
                        issue_hcg(g)
                for k in range(6):
                    stagesAB(k, k - 1)
                for t in range(T):
                    if t % XGG == 0 and t > 0:
                        issue_gather(t // XGG + 1)
                    issue_selch(((t + 4) * m) // SELCH)
                    if leafch:
                        issue_hcg((2 * t + 27) // HCG)
                    stagesAB(t + 6, t + 5)
                    stageC(t)
                    if t >= 1:
                        stageDE(t - 1)
                stageDE(T - 1)
                state[d] = (h_lv, c_lv)

            # ---- final linear on the roots ----
            h0 = state[0][0][0][:, 0, :]
            hrT = transpose256(h0, "hrT")
            plin = p256_pool.tile([128, 128], F32, tag="p256", name="plin")
            pairs = [(lin_sb[:, 0, :], hrT[:, 0:128]),
                     (lin_sb[:, 1, :], hrT[:, 128:256])]
            if with_linb:
                pairs.append((bias_sb[:, 4 * H:4 * H + NCLS], ones_sb[:]))
            matmul_group(plin[0:NCLS, :], pairs)
            out_sb = cpool.tile([128, 128], F32)
            nc.vector.tensor_copy(out_sb[0:NCLS, :], plin[0:NCLS, :])
            nc.sync.dma_start(logits_t[:], out_sb[0:NCLS, :])

    nc.compile()
    return nc


# --------------------------------------------------------------------------
# host wrapper
# --------------------------------------------------------------------------

def prepare(inputs):
    """Returns ((Ls, Ts, ms, flags), in_maps)."""
    x = np.asarray(inputs["x"]).astype(np.int64)
    par = np.asarray(inputs["par"]).astype(np.int64)
    emb = np.asarray(inputs["emb"], dtype=np.float32).copy()
    emb[PAD] = 0.0
    W = np.asarray(inputs["W_iou"], np.float32).copy()
    U = np.asarray(inputs["U_iou"], np.float32).copy()
    Uf = np.asarray(inputs["U_f_w"], np.float32)
    lin = np.asarray(inputs["lin_w"], np.float32)
    b_iou = np.asarray(inputs["b_iou"], np.float32).reshape(-1).copy()
    ufb = np.asarray(inputs["U_f_b"], np.float32).reshape(-1)
    linb = np.asarray(inputs["lin_b"], np.float32).reshape(-1)

    # ---- leaf folding: a leaf's (h, c) depends only on its token, so the
    # whole leaf-level LSTM cell is precomputed per vocab entry on the host.
    # Table rows are [h_leaf | c_leaf/2] (c~ = c/2 is what the device
    # tracks).
    iou_leaf = emb @ W.T + b_iou[None, :]
    il, ol, ul = (iou_leaf[:, 0:H], iou_leaf[:, H:2 * H],
                  iou_leaf[:, 2 * H:3 * H])
    sig = lambda v: 1.0 / (1.0 + np.exp(-v))  # noqa: E731
    c_leaf = sig(il) * np.tanh(ul)
    h_leaf = sig(ol) * np.tanh(c_leaf)
    hc8 = np.concatenate([h_leaf, 0.5 * c_leaf], axis=1).astype(np.float16)
    # leaf h again as a scaled fp8 table for the transposed gather feeding
    # the U_f DoubleRow lhsT (the 16-bit-granularity transpose interleaves
    # fp8 pairs: partition p holds dims (2p, 2p+1))
    ht8 = (h_leaf * SH).astype(ml_dtypes.float8_e4m3)

    # fold the x @ W_iou^T product into the embedding table; pre-scale the
    # u-slice by 2 (the device tracks c~ = c/2 and computes
    # tanh(u) via 2*sigmoid(2u) - 1)
    W[2 * H:3 * H] *= 2.0
    U[2 * H:3 * H] *= 2.0
    b_iou[2 * H:3 * H] *= 2.0
    # U_iou / U_f run as fp8e4m3 DoubleRow matmuls.  fp8's dynamic range
    # bottoms out near these weights' natural ~0.05 magnitude, so the
    # weights carry a x SU scale and the transposed-h operands a x SH
    # scale; the iou/f sigmoids read PSUM with scale 1/(SU*SH).  The
    # x-side preacts (emb8) and biases are pre-scaled to match.
    # column order [i | u | o]: the u-slice rides next to i so the tail
    # levels can sigmoid the chain-critical (i, u) pair first
    iuo = np.concatenate([np.arange(0, H), np.arange(2 * H, 3 * H),
                          np.arange(H, 2 * H)])
    emb8 = (emb @ W.T * SCALE)[:, iuo].astype(np.float16)

    lay = build_layouts(x, par)
    Ls, Ts, ms = lay["L"], lay["T"], lay["m"]

    uiou8T = np.ascontiguousarray(
        (U.T * SCALE)[:, iuo].reshape(2, 128, 3 * H)).astype(np.float16)
    ufT = np.ascontiguousarray(Uf.T.reshape(2, 128, H) * SU).astype(
        ml_dtypes.float8_e4m3)
    # interleaved-row variant matching the fp8 transposed-gather layout:
    # (p, j) holds U_f^T row 2p+j
    ufT_il = np.ascontiguousarray(Uf.T.reshape(128, 2, H) * SU).astype(
        ml_dtypes.float8_e4m3)
    linT = np.ascontiguousarray(lin.T.reshape(2, 128, NCLS)).astype(
        np.float16)
    biases = np.concatenate([(b_iou * SCALE)[iuo], ufb * SCALE,
                             linb]).astype(np.float16)[None, :]

    flags = dict(with_biou=bool(np.any(b_iou)), with_ufb=bool(np.any(ufb)),
                 with_linb=bool(np.any(linb)))

    in_maps = []
    for c in range(NCORES):
        cl = lay["cores"][c]
        xtokc = np.concatenate([_wrap16(cl["xtok"][d]) for d in range(DEPTH)],
                               axis=1)
        xtokc = np.tile(xtokc, (8, 1))  # replicate across the 8 Q7 cores
        nsel = sum(Ts[d] * ms[d] for d in range(DEPTH - 1))
        rel = np.concatenate(
            [cl["relslot"][d].reshape(-1, 128) for d in range(DEPTH - 1)],
            axis=0)  # [nsel, 128] float16 rel slot per (sel, child-part)
        # one-hot selector matrices, laid out [child_part, sel, parent_slot]
        sel1h = (rel[:, :, None] ==
                 np.arange(128, dtype=np.float32)[None, None, :])
        selm = np.ascontiguousarray(
            sel1h.transpose(1, 0, 2)).astype(np.float16)
        if nsel == 0:
            selm = np.zeros((128, 1, 128), np.float16)
        in_maps.append({
            "emb8": emb8,
            "hc8": hc8,
            "ht8": ht8,
            "uiou8T": uiou8T,
            "ufT": ufT,
            "ufT_il": ufT_il,
            "linT": linT,
            "biases": biases,
            "xtok": np.ascontiguousarray(xtokc).astype(np.int16),
            "selmat": selm,
        })
    return (Ls, Ts, ms, flags), in_maps


_PROGRAM_CACHE = {}


def get_program(Ls, Ts, ms, flags):
    key = (tuple(Ls), tuple(ms), tuple(sorted(flags.items())))
    if key not in _PROGRAM_CACHE:
        _PROGRAM_CACHE[key] = build_program(Ls, Ts, ms, **flags)
    return _PROGRAM_CACHE[key]


def _ensure_device_backend():
    """run_bass_kernel_spmd executes via jax.devices(); make sure those are
    the NeuronCores, not a host-emulation platform."""
    import jax
    try:
        if jax.devices()[0].platform in ("neuron", "axon"):
            return
    except Exception:
        pass
    try:
        jax.config.update("jax_platforms", "neuron")
        jax.clear_backends()
        assert jax.devices()[0].platform in ("neuron", "axon")
    except Exception:
        pass


def kernel(**inputs):
    _ensure_device_backend()
    (Ls, Ts, ms, flags), in_maps = prepare(inputs)
    nc = get_program(Ls, Ts, ms, flags)
    res = run_bass_kernel_spmd(nc, in_maps, core_ids=list(range(NCORES)))
    logits = np.zeros((B, NCLS), np.float32)
    for c in range(NCORES):
        lt = res.results[c]["logits_t"]  # [104, 128]
        logits[c * TPC:(c + 1) * TPC] = np.asarray(lt)[:, 0:TPC].T
    return logits

